# revision 9
# baseline (speedup 1.0000x reference)
"""GAT message-passing kernel for 8 Trainium2 NeuronCores.

Strategy (dst-sharded padded-CSR, no collectives):
  - Host: shard nodes into 8 contiguous ranges balanced by in-edge count.
    Edges follow their dst node; each core computes output rows for its own
    nodes only, so no cross-core reduction is needed.
  - Device, per core:
      Phase 1: project features -> per-node table rows [feat(64) | el(4) |
               junk] (256B rows) written to a DRAM table; el folded into the
               projection matmul via an augmented weight matrix. The host
               permutes fT columns so that matmul j emits, on partition p, the
               record of table row p*NT+j: each partition then holds a
               CONTIGUOUS run of table rows and the table write needs only
               128 large descriptors per slab (vs 1 per 256B row).
               A small second pass computes er for the core's own nodes.
      Phase 2: for each tile of 128 dst nodes, dma_gather the fat table rows
               of their (padded) src neighbor lists, compute
               ee = exp(leaky_relu(el_src + er_dst)) on DVE/ACT writing ee
               (bf16) back into the gathered rows' el slot, multiply feats by
               ee, and do ONE fused segment-reduce over [128, 68, C] that
               yields both the weighted feat sum and the ee sum.
               Softmax normalization is deferred: out = (sum ee*feat)/(sum ee).
  - int16 gather indices can't address 50000 rows, so the table is split at
    row 32768 (A = rows < 32768, B = rest) and each node's neighbor list is
    split into A/B sublists; nodes are tiled grouped by A-degree (B-pass uses
    an independent B-degree ordering plus an on-device combine gather).
    Dummy rows with el=-1e30 make padded slots contribute exactly zero.
"""

import os
import sys
from contextlib import ExitStack

import numpy as np

sys.path.insert(0, "/opt/trn_rl_repo")

# ---------------- problem constants (hardcoded per spec) ----------------
N_NODES = 50000
N_EDGES = 1600000
IN_DIM = 128
HEADS = 4
HID = 16
FEAT = HEADS * HID  # 64
NEG_SLOPE = 0.2
NCORES = 8
P = 128
ROW = 128          # table row size in bf16 elems (256 B)
EL_OFF = 64        # el lives at row[64:68] (f32 in bf16 elems 64:72)
NEG_BIG = -1.0e30

# table layout: [A nodes | A dummies][B nodes | B pad | B dummies]
A_CAP = 32768      # int16 addressing limit for the A side
NDUM = 512         # dummy rows per side
A_NODES = A_CAP - NDUM          # 32256 real nodes on the A side
NT_A = A_NODES // P             # 252 rows per partition chunk (A region)

# tunables
CMAX = int(os.environ.get("GAT_CMAX", "32"))   # max gather cols/unit
SLAB_TILES = int(os.environ.get("GAT_SLAB", "32"))  # node-tiles per p1 slab
NQUEUES = int(os.environ.get("GAT_NQUEUES", "4"))
DMA_SCRATCH = int(os.environ.get("GAT_SCRATCH", "16384"))
# a gather call of ck cols emits ~8*ck+1 tx descs and must fit one SDMA
# packet (<=64 data descs) => ck <= 8 per call with single_packet
CALLMAX = int(os.environ.get("GAT_CALLMAX", "8"))
CHAIN = os.environ.get("GAT_CHAIN", "1") == "1"
DEPTH = int(os.environ.get("GAT_DEPTH", "2"))  # in-flight gather calls/queue
GBUFS = int(os.environ.get("GAT_GBUFS", "4"))
SKIP_P2 = os.environ.get("GAT_SKIP_P2", "0") == "1"
SKIP_GATHER = os.environ.get("GAT_SKIP_GATHER", "0") == "1"
SKIP_COMPUTE = os.environ.get("GAT_SKIP_COMPUTE", "0") == "1"


def _round_up(x, m):
    return (x + m - 1) // m * m


# ---------------- host-side planning ----------------

class Plan:
    pass


def make_plan(src, dst, n_nodes, n_edges, ncores, cmax=CMAX):
    """Pure-index planning: core shards, node order, tile schedule, gather
    index arrays, table row mapping."""
    src = np.asarray(src).astype(np.int64)
    dst = np.asarray(dst).astype(np.int64)
    N = n_nodes
    E = n_edges

    deg = np.bincount(dst, minlength=N)
    cum = np.cumsum(deg)
    # node-id boundaries for the 8 cores, balanced by edge count
    targets = (np.arange(1, ncores) * E) // ncores
    inner = np.searchsorted(cum, targets, side="left") + 1
    bnds = np.concatenate([[0], inner, [N]]).astype(np.int64)
    Lc = np.diff(bnds)
    assert (Lc > 0).all()

    # ---- table row layout ----
    # A region: rows [0, A_NODES) in 128 partition-chunks of NT_A rows,
    #           A dummies [A_NODES, A_CAP)
    # B region: rows [A_CAP, A_CAP + 128*NT_B) (incl pad), B dummies after
    NB = N - A_NODES
    NT_B = _round_up(NB, P) // P          # rows per partition chunk (B)
    B_BASE = A_CAP
    B_ROWS = P * NT_B
    dummyB0 = B_BASE + B_ROWS
    TROWS = dummyB0 + NDUM
    assert B_ROWS + NDUM <= 32767, "B side exceeds int16 range"

    # node -> table row. Nodes 0..A_NODES-1 go to region A, rest to B.
    # Within a region, node k (0-based within region) -> partition-chunk
    # layout row: base + (k % P) * NT + (k // P). This makes the j-th p1
    # matmul (which processes nodes with k//P == j, k%P == p on partition p)
    # emit records that are contiguous-per-partition.
    nid = np.arange(N)
    k_a = nid  # region-local index for A nodes (only valid < A_NODES)
    rowA = (k_a % P) * NT_A + k_a // P
    k_b = nid - A_NODES
    rowB = B_BASE + (k_b % P) * NT_B + k_b // P
    tblrow = np.where(nid < A_NODES, rowA, rowB)
    # fT column order: column j*128+p must hold the node at region row
    # p*NT+j. Build inverse mapping: ft_node[col] = node id (or -1 for pad).
    ft_cols_A = np.full(P * NT_A, -1, dtype=np.int64)
    ft_cols_A[(k_a[:A_NODES] % P) * NT_A + k_a[:A_NODES] // P] = nid[:A_NODES]
    # stored column-major per matmul: col j*128+p <-> region row p*NT+j
    colsA = np.full((NT_A, P), -1, dtype=np.int64)
    colsA[k_a[:A_NODES] // P, k_a[:A_NODES] % P] = nid[:A_NODES]
    colsB = np.full((NT_B, P), -1, dtype=np.int64)
    colsB[k_b[A_NODES:] // P, k_b[A_NODES:] % P] = nid[A_NODES:]
    ft_node = np.concatenate([colsA.reshape(-1), colsB.reshape(-1)])

    isA = src < A_NODES
    a_deg = np.bincount(dst[isA], minlength=N)
    b_deg = deg - a_deg

    # per-core node ordering: A-pass tiles sorted by -a_deg, B-pass tiles
    # independently sorted by -b_deg; positions within core: slot-major
    T = int(max((Lc + P - 1) // P))
    L = T * P
    order = np.full((ncores, L), -1, dtype=np.int64)
    orderB = np.full((ncores, L), -1, dtype=np.int64)
    posl_a = np.empty(N, dtype=np.int64)  # within-core A-order position
    for c in range(ncores):
        ids = np.arange(bnds[c], bnds[c + 1])
        o = ids[np.lexsort((-b_deg[ids], -a_deg[ids]))]
        order[c, : len(o)] = o
        posl_a[o] = np.arange(len(o))
        ob = ids[np.lexsort((-a_deg[ids], -b_deg[ids]))]
        orderB[c, : len(ob)] = ob

    # edges sorted by (dst, then A-first)
    eorder = np.lexsort((~isA, dst))
    s_sorted = src[eorder]
    tid_sorted = tblrow[s_sorted]            # table row of each edge's src
    seg_start = np.concatenate([[0], cum])   # per-dst segment starts

    # per-(core,tile) max a/b degree -> global schedule (per-pass orders)
    KA = np.zeros((ncores, T), dtype=np.int64)
    KB = np.zeros((ncores, T), dtype=np.int64)
    for c in range(ncores):
        for t in range(T):
            sa = order[c, t * P : (t + 1) * P]
            sa = sa[sa >= 0]
            if len(sa):
                KA[c, t] = a_deg[sa].max()
            sb = orderB[c, t * P : (t + 1) * P]
            sb = sb[sb >= 0]
            if len(sb):
                KB[c, t] = b_deg[sb].max()
    KAg = KA.max(axis=0)
    KBg = KB.max(axis=0)

    # units: per tile-pass, chunks <= cmax cols, calls <= CALLMAX cols.
    # All A-units of all tiles run before any B-unit so the B-half of the
    # table build overlaps under the A gathers.
    def _side_units(t, side, k):
        out = []
        k0 = 0
        while k0 < k:
            cur = {"t": t, "side": side, "calls": [], "cols": 0}
            while cur["cols"] < cmax and k0 < k:
                ck = min(cmax - cur["cols"], k - k0, CALLMAX)
                cur["calls"].append((side, cur["cols"], k0, ck))
                cur["cols"] += ck
                k0 += ck
            out.append(cur)
        return out

    a_units, b_units = [], []
    for t in range(T):
        ua = _side_units(t, "A", int(KAg[t]))
        ub = _side_units(t, "B", int(KBg[t]))
        if not ua:
            ua = [{"t": t, "side": "A", "calls": [], "cols": 0}]
        if not ub:
            ub = [{"t": t, "side": "B", "calls": [], "cols": 0}]
        for u in ua + ub:
            u["first"] = False
            u["last"] = False
        ua[0]["first"] = True
        ua[-1]["alast"] = True
        ub[0]["first"] = True
        ub[-1]["last"] = True
        a_units.extend(ua)
        b_units.extend(ub)
    units = a_units + b_units

    # per-core int16 index arrays, packed per unit/call in 16-partition wrap
    totcols = 8 * sum(u["cols"] for u in units)
    totcols = max(totcols, 8)
    idx_arr = np.zeros((ncores, 16, totcols), dtype=np.int16)
    call_off = []
    off = 0
    for u in units:
        offs = []
        for side, gcol, k0, ck in u["calls"]:
            offs.append(off)
            off += 8 * ck
        call_off.append(offs)

    rngp = np.random.default_rng(12345)
    for c in range(ncores):
        colA_cache = {}
        colB_cache = {}
        for t in range(T):
            slots = order[c, t * P : (t + 1) * P]
            ok = slots >= 0
            sl = np.where(ok, slots, 0)
            al = np.where(ok, a_deg[sl], 0).astype(np.int64)
            st = seg_start[sl]
            slotsB = orderB[c, t * P : (t + 1) * P]
            okB = slotsB >= 0
            slB = np.where(okB, slotsB, 0)
            alB = np.where(okB, a_deg[slB], 0).astype(np.int64)
            bl = np.where(okB, b_deg[slB], 0).astype(np.int64)
            stB = seg_start[slB]
            ka, kb = int(KAg[t]), int(KBg[t])
            if ka > 0:
                arr = A_NODES + rngp.integers(0, NDUM, size=(P, ka))
                tot = int(al.sum())
                if tot:
                    cs = np.concatenate([[0], np.cumsum(al)[:-1]])
                    flat = np.repeat(st, al) + (np.arange(tot) - np.repeat(cs, al))
                    mask = np.arange(ka)[None, :] < al[:, None]
                    arr[mask] = tid_sorted[flat]
                colA_cache[t] = arr.T  # [ka, 128]
            if kb > 0:
                # B-relative rows: dummies at [dummyB0-B_BASE, +NDUM)
                arr = (dummyB0 - B_BASE) + rngp.integers(0, NDUM, size=(P, kb))
                tot = int(bl.sum())
                if tot:
                    cs = np.concatenate([[0], np.cumsum(bl)[:-1]])
                    flat = (np.repeat(stB + alB, bl)
                            + (np.arange(tot) - np.repeat(cs, bl)))
                    mask = np.arange(kb)[None, :] < bl[:, None]
                    arr[mask] = tid_sorted[flat] - B_BASE
                colB_cache[t] = arr.T
        for ui, u in enumerate(units):
            for (side, gcol, k0, ck), o in zip(u["calls"], call_off[ui]):
                cols = (colA_cache if side == "A" else colB_cache)[u["t"]]
                blk = cols[k0 : k0 + ck].reshape(-1)  # k-major flat, 128*ck
                assert blk.max() <= 32767 and blk.min() >= 0
                idx_arr[c, :, o : o + 8 * ck] = (
                    blk.reshape(-1, 16).T.astype(np.int16))

    idx_full = np.tile(idx_arr, (1, 8, 1))  # [ncores, 128, totcols]

    # combine gather: accA rows (within-core A-order) for each B slot,
    # packed k-major over tiles in chunks of 8 tiles (1024 idxs/call)
    CCH = 8
    ncomb = (T + CCH - 1) // CCH
    comb_cols = ncomb * CCH * 8
    comb_arr = np.zeros((ncores, 16, comb_cols), dtype=np.int16)
    for c in range(ncores):
        qa = np.zeros((T, P), dtype=np.int64)
        for t in range(T):
            sb = orderB[c, t * P : (t + 1) * P]
            okb = sb >= 0
            slot = posl_a[sb[okb]]
            # accAd rows are slot-major: row = (slot%128)*T + slot//128
            qa[t, okb] = (slot % P) * T + slot // P
        for j in range(ncomb):
            blk = qa[j * CCH : (j + 1) * CCH]
            if blk.shape[0] < CCH:
                blk = np.concatenate(
                    [blk, np.zeros((CCH - blk.shape[0], P), np.int64)])
            flat = blk.reshape(-1)
            comb_arr[c, :, j * CCH * 8 : (j + 1) * CCH * 8] = (
                flat.reshape(-1, 16).T.astype(np.int16))
    comb_full = np.tile(comb_arr, (1, 8, 1))

    p = Plan()
    p.N, p.E, p.ncores = N, E, ncores
    p.T, p.L, p.TROWS = T, L, TROWS
    p.NB, p.NT_B, p.B_BASE, p.B_ROWS = NB, NT_B, B_BASE, B_ROWS
    p.dummyB0 = dummyB0
    p.order, p.orderB, p.tblrow, p.ft_node = order, orderB, tblrow, ft_node
    p.comb_full, p.comb_cols, p.CCH = comb_full, comb_cols, CCH
    p.units, p.call_off, p.totcols = units, call_off, totcols
    p.idx_full = idx_full
    p.KAg, p.KBg = KAg, KBg
    return p


# ---------------- device kernel builder ----------------

def build_nc(plan, num_cores, slab_tiles=SLAB_TILES, cmax=CMAX, reps=1):
    import concourse.bacc as bacc
    import concourse.bass as bass
    import concourse.tile as tile
    from concourse import mybir
    from concourse.tile import add_dep_helper

    f32 = mybir.dt.float32
    bf16 = mybir.dt.bfloat16
    i16 = mybir.dt.int16
    Alu = mybir.AluOpType
    Act = mybir.ActivationFunctionType

    T, TROWS = plan.T, plan.TROWS
    totcols = plan.totcols
    NT_B = plan.NT_B
    NFT = NT_A + NT_B  # fT matmuls (table-build tiles)

    nc = bacc.Bacc("TRN2", target_bir_lowering=False, debug=False,
                   enable_asserts=False, num_devices=num_cores,
                   num_swdge_queues=NQUEUES,
                   dynamic_dma_scratch_size=DMA_SCRATCH)

    fT = nc.dram_tensor("fT", [P, NFT * P], bf16, kind="ExternalInput").ap()
    fLT = nc.dram_tensor("fLT", [P, T * P], bf16, kind="ExternalInput").ap()
    Waug = nc.dram_tensor("Waug", [P, 68], bf16, kind="ExternalInput").ap()
    Wr = nc.dram_tensor("Wr", [P, 4], bf16, kind="ExternalInput").ap()
    idx = nc.dram_tensor("idx", [P, totcols], i16, kind="ExternalInput").ap()
    comb = nc.dram_tensor("comb", [P, plan.comb_cols], i16,
                          kind="ExternalInput").ap()
    fLTB = nc.dram_tensor("fLTB", [P, T * P], bf16, kind="ExternalInput").ap()
    biasm = nc.dram_tensor("biasm", [1, 16], f32, kind="ExternalInput").ap()
    out = nc.dram_tensor("out", [T * P, 16], f32, kind="ExternalOutput").ap()

    with tile.TileContext(nc) as tc, ExitStack() as ctx:
        dpool = ctx.enter_context(tc.tile_pool(name="dram", bufs=1, space="DRAM"))
        wpool = ctx.enter_context(tc.tile_pool(name="wpool", bufs=1))
        nping = 2 if reps > 1 else 1
        tables = [dpool.tile([TROWS, ROW], bf16, name=f"table{b}")
                  for b in range(nping)]
        accAd = dpool.tile([T * P, ROW], bf16, name="accAd")
        erA_sbs = [wpool.tile([P, T, 4], f32, tag=f"erA{b}", name=f"erA{b}")
                   for b in range(nping)]
        erB_sbs = [wpool.tile([P, T, 4], f32, tag=f"erB{b}", name=f"erB{b}")
                   for b in range(nping)]
        accA_all = wpool.tile([P, T, 68], f32, tag="accA", name="accA")
        accB_all = wpool.tile([P, T, 68], f32, tag="accB", name="accB")
        accAbf = wpool.tile([P, T, ROW], bf16, tag="accAbf", name="accAbf")
        nc.vector.memset(accAbf[:], 0.0)
        ncomb = plan.comb_cols // 64
        G2 = wpool.tile([P, ncomb * plan.CCH, ROW], bf16, tag="G2", name="G2")
        outbuf = wpool.tile([P, T, 16], f32, tag="outbuf", name="outbuf")

        # static tiles, loaded once (weights/topology do not change per rep)
        waug_sb = wpool.tile([P, 68], bf16, tag="waug")
        wr_sb = wpool.tile([P, 4], bf16, tag="wr")
        biasm_sb = wpool.tile([P, 16], f32, tag="biasm")
        idx_sb = wpool.tile([P, totcols], i16, tag="idx")
        comb_sb = wpool.tile([P, plan.comb_cols], i16, tag="comb")
        nc.sync.dma_start(comb_sb[:], comb)
        nc.sync.dma_start(waug_sb[:], Waug)
        nc.sync.dma_start(wr_sb[:], Wr)
        nc.sync.dma_start(biasm_sb[:1, :], biasm)
        nc.gpsimd.partition_broadcast(biasm_sb[:], biasm_sb[:1, :])
        nc.sync.dma_start(idx_sb[:], idx)

        # dummy rows: el = -1e30 (feat junk is harmless: ee = 0 kills it).
        # Dummies are never overwritten by the per-rep table build, so init
        # them once. A dummies: rows [A_NODES, A_CAP); B: [dummyB0, TROWS).
        neg_sb = wpool.tile([P, NDUM // P, ROW], bf16, tag="neg")
        nc.vector.memset(neg_sb[:], 0.0)
        nc.vector.memset(
            neg_sb[:, :, EL_OFF : EL_OFF + 8].bitcast(f32), NEG_BIG)
        for b in range(nping):
            for d0 in (A_NODES, plan.dummyB0):
                nc.sync.dma_start(
                    tables[b][d0 : d0 + NDUM, :]
                    .rearrange("(j p) f -> p j f", p=P),
                    neg_sb[:])

        # transient pools shared by both ping-pong bodies
        slabp = ctx.enter_context(tc.tile_pool(name="slab", bufs=2))
        stagep = ctx.enter_context(tc.tile_pool(name="stage", bufs=2))
        psp = ctx.enter_context(tc.tile_pool(name="ps1", bufs=4, space="PSUM"))
        gp = ctx.enter_context(tc.tile_pool(name="gp", bufs=GBUFS))
        eep = ctx.enter_context(tc.tile_pool(name="eep", bufs=4))
        tmpp = ctx.enter_context(tc.tile_pool(name="tmpp", bufs=6))

        qstate = {"qn": 0, "last": {}}

        def emit_p1(b):
            table = tables[b]

            # er for the core's own nodes (A order and B order)
            for src_t, er_dst in ((fLT, erA_sbs[b]), (fLTB, erB_sbs[b])):
                fl = slabp.tile([P, T * P], bf16, tag="fl")
                nc.sync.dma_start(fl[:], src_t)
                for t in range(T):
                    pse = psp.tile([P, 4], f32, tag="pse")
                    nc.tensor.matmul(out=pse[:],
                                     lhsT=fl[:, t * P : (t + 1) * P],
                                     rhs=wr_sb[:], start=True, stop=True)
                    nc.vector.tensor_copy(er_dst[:, t, :], pse[:])

            # table build: region A is fT tiles [0, NT_A), region B the rest.
            # fT column j*128+p holds the node of table row base + p*NT + j,
            # so partition p's stage row j is table row p*NT+j: the write is
            # one contiguous (nt*256B) descriptor per partition.
            def build_region(base_row, nt_region, ft_lo):
                t0 = 0
                while t0 < nt_region:
                    nt = min(slab_tiles, nt_region - t0)
                    slab = slabp.tile([P, slab_tiles * P], bf16, tag="slab")
                    nc.sync.dma_start(
                        slab[:, : nt * P],
                        fT[:, (ft_lo + t0) * P : (ft_lo + t0 + nt) * P])
                    stage = stagep.tile([P, slab_tiles, ROW], bf16,
                                        tag="stage")
                    for j in range(nt):
                        ps = psp.tile([P, 68], f32, tag="ps")
                        nc.tensor.matmul(out=ps[:],
                                         lhsT=slab[:, j * P : (j + 1) * P],
                                         rhs=waug_sb[:], start=True, stop=True)
                        nc.scalar.activation(stage[:, j, 0:64], ps[:, 0:64],
                                             Act.Copy)
                        nc.scalar.activation(
                            stage[:, j, 64:72].bitcast(f32), ps[:, 64:68],
                            Act.Copy)
                    # partition p -> rows [base + p*nt_region + t0, +nt);
                    # writes ride the ACT HWDGE ring so they don't block the
                    # next slab load (SP ring) behind their stage dependency
                    dst = table[base_row : base_row + P * nt_region, :] \
                        .rearrange("(p j) f -> p j f", p=P)
                    nc.scalar.dma_start(
                        dst[:, t0 : t0 + nt, :], stage[:, :nt, :])
                    t0 += nt

            build_region(0, NT_A, 0)
            build_region(plan.B_BASE, NT_B, NT_A)

        def emit_p2(b):
            table = tables[b]
            tabA = table[0:A_CAP, :]
            tabB = table[plan.B_BASE : TROWS, :]
            flushed = [False]
            for ui, u in enumerate(plan.units):
                if SKIP_P2:
                    break
                t = u["t"]
                C = u["cols"]
                isA_u = u["side"] == "A"
                if not isA_u and not flushed[0]:
                    # end of A pass: flush all A accumulators (cast f32->bf16)
                    # to DRAM in one contiguous slot-major DMA for the combine
                    flushed[0] = True
                    if not (SKIP_GATHER or SKIP_COMPUTE):
                        nc.vector.tensor_copy(accAbf[:, :, 0:68], accA_all[:])
                    nc.scalar.dma_start(
                        accAd.rearrange("(p t) f -> p t f", p=P), accAbf[:])
                if not isA_u and u["first"] and t % plan.CCH == 0:
                    # A accumulators (bf16 rows in accAd) -> B slot order
                    j = t // plan.CCH
                    gi = nc.gpsimd.dma_gather(
                        G2[:, j * plan.CCH : (j + 1) * plan.CCH, :],
                        accAd,
                        comb_sb[:, j * 8 * plan.CCH : (j + 1) * 8 * plan.CCH],
                        plan.CCH * P, plan.CCH * P, ROW,
                        queue_num=qstate["qn"])
                    if CHAIN:
                        hist = qstate["last"].setdefault(qstate["qn"], [])
                        hist.append(gi.ins)
                        if len(hist) > DEPTH:
                            add_dep_helper(gi.ins, hist.pop(0),
                                           reason="swdge ring throttle")
                    qstate["qn"] = (qstate["qn"] + 1) % NQUEUES
                er_sb = erA_sbs[b] if isA_u else erB_sbs[b]
                acc_all = accA_all if isA_u else accB_all
                acc68 = acc_all[:, t, 0:68]

                def do_gathers(G):
                    for (side, gcol, k0, ck), o in zip(u["calls"],
                                                       plan.call_off[ui]):
                        tab = tabA if side == "A" else tabB
                        gi = nc.gpsimd.dma_gather(
                            G[:, gcol : gcol + ck, :], tab,
                            idx_sb[:, o : o + 8 * ck],
                            ck * P, ck * P, ROW, queue_num=qstate["qn"])
                        if CHAIN:
                            hist = qstate["last"].setdefault(qstate["qn"], [])
                            hist.append(gi.ins)
                            if len(hist) > DEPTH:
                                add_dep_helper(gi.ins, hist.pop(0),
                                               reason="swdge ring throttle")
                        qstate["qn"] = (qstate["qn"] + 1) % NQUEUES

                if SKIP_GATHER or SKIP_COMPUTE:
                    G = gp.tile([P, cmax, ROW], bf16, tag="G")
                    if not SKIP_GATHER:
                        do_gathers(G)
                    if u["last"] and not isA_u:
                        fin = tmpp.tile([P, 16], f32, tag="fin")
                        if SKIP_GATHER:
                            nc.vector.memset(fin[:], 0.0)
                        else:
                            nc.vector.tensor_reduce(
                                fin[:], G[:, :max(C, 1), 0:16],
                                mybir.AxisListType.XY, Alu.add)
                        nc.vector.tensor_copy(outbuf[:, t, :], fin[:])
                    continue
                if C > 0:
                    G = gp.tile([P, cmax, ROW], bf16, tag="G")
                    do_gathers(G)
                    ee = eep.tile([P, cmax, 4], f32, tag="ee")
                    # e = el_src + er_dst
                    nc.vector.tensor_tensor(
                        ee[:, :C, :],
                        G[:, :C, EL_OFF : EL_OFF + 8].bitcast(f32),
                        er_sb[:, t : t + 1, :].to_broadcast([P, C, 4]),
                        Alu.add)
                    # leaky relu: max(x, 0.2x)
                    nc.vector.scalar_tensor_tensor(
                        ee[:, :C, :], ee[:, :C, :], NEG_SLOPE, ee[:, :C, :],
                        Alu.mult, Alu.max)
                    # exp -> bf16, written into the gathered rows' el slot so
                    # one reduce covers both msg and denominator
                    nc.scalar.activation(G[:, :C, EL_OFF : EL_OFF + 4],
                                         ee[:, :C, :], Act.Exp)
                    # msg = feat * ee (broadcast over hid dim)
                    nc.vector.tensor_tensor(
                        G[:, :C, 0:FEAT].rearrange("p c (h d) -> p c h d",
                                                   h=HEADS),
                        G[:, :C, 0:FEAT].rearrange("p c (h d) -> p c h d",
                                                   h=HEADS),
                        G[:, :C, EL_OFF : EL_OFF + 4].unsqueeze(-1)
                        .to_broadcast([P, C, HEADS, HID]),
                        Alu.mult)
                    # fused k-reduction: [128, 68, C] -> msg sum | ee sum
                    red = G[:, :C, 0:68].rearrange("p c f -> p f c")
                    if u["first"]:
                        nc.vector.tensor_reduce(acc68, red,
                                                mybir.AxisListType.X, Alu.add)
                    else:
                        tmp = tmpp.tile([P, 68], f32, tag="tmp")
                        nc.vector.tensor_reduce(tmp[:], red,
                                                mybir.AxisListType.X, Alu.add)
                        nc.vector.tensor_add(acc68, acc68, tmp[:])
                elif u["first"]:
                    nc.vector.memset(acc68, 0.0)
                if u["last"] and not isA_u:
                    tmp68 = tmpp.tile([P, 68], f32, tag="tmp68")
                    nc.vector.tensor_tensor(
                        tmp68[:], G2[:, t, 0:68], acc68, Alu.add)
                    dent = tmp68[:, FEAT : FEAT + 4]
                    nc.vector.tensor_scalar_max(dent, dent, 1e-30)
                    rec = tmpp.tile([P, 4], f32, tag="rec")
                    nc.vector.reciprocal(rec[:], dent)
                    nc.vector.tensor_tensor(
                        tmp68[:, 0:FEAT].rearrange("p (h d) -> p h d",
                                                   h=HEADS),
                        tmp68[:, 0:FEAT].rearrange("p (h d) -> p h d",
                                                   h=HEADS),
                        rec[:].unsqueeze(-1).to_broadcast([P, HEADS, HID]),
                        Alu.mult)
                    fin = tmpp.tile([P, 16], f32, tag="fin")
                    nc.vector.tensor_reduce(
                        fin[:],
                        tmp68[:, 0:FEAT].rearrange("p (h d) -> p d h",
                                                   h=HEADS),
                        mybir.AxisListType.X, Alu.add)
                    nc.vector.scalar_tensor_tensor(
                        outbuf[:, t, :], fin[:], 1.0 / HEADS, biasm_sb[:],
                        Alu.mult, Alu.add)
            if not SKIP_P2:
                # slot-major output: out row p*T + t <- outbuf[p, t, :]
                nc.sync.dma_start(
                    out.rearrange("(p t) f -> p t f", p=P), outbuf[:])

        if reps > 1:
            R = reps - (reps % 2)
            emit_p1(0)
            if R // 2 - 1 > 0:
                with tc.For_i(0, R // 2 - 1, 1):
                    emit_p1(1)
                    emit_p2(0)
                    emit_p1(0)
                    emit_p2(1)
            emit_p1(1)
            emit_p2(0)
            emit_p2(1)
            if reps % 2:
                emit_p1(0)
                emit_p2(0)
        else:
            emit_p1(0)
            emit_p2(0)

    nc.compile()
    return nc


# ---------------- host wrapper ----------------

_CACHE = {}


def _get_plan_and_nc(src, dst, n_nodes, n_edges, ncores, cmax=CMAX,
                     slab_tiles=SLAB_TILES):
    key = (int(src[0]), int(src[-1]), int(dst[0]), int(dst[-1]),
           len(src), n_nodes, ncores, cmax)
    if key not in _CACHE:
        plan = make_plan(src, dst, n_nodes, n_edges, ncores, cmax=cmax)
        nc = build_nc(plan, ncores, slab_tiles=slab_tiles, cmax=cmax)
        _CACHE[key] = (plan, nc)
    return _CACHE[key]


def make_inputs(plan, features, W, attn_l, attn_r, bias):
    """Build per-core input maps from full inputs + plan."""
    import ml_dtypes

    bf16 = ml_dtypes.bfloat16
    features = np.asarray(features, dtype=np.float32)
    W = np.asarray(W, dtype=np.float32)
    attn_l = np.asarray(attn_l, dtype=np.float32)
    attn_r = np.asarray(attn_r, dtype=np.float32)
    bias = np.asarray(bias, dtype=np.float32)

    # augmented weights
    W3 = W.reshape(IN_DIM, HEADS, HID)
    Wl = np.einsum("ihd,hd->ih", W3, attn_l).astype(np.float32)
    Wr_ = np.einsum("ihd,hd->ih", W3, attn_r).astype(bf16)
    Waug = np.concatenate([W, Wl], axis=1).astype(bf16)  # [128, 68]
    biasm = bias.reshape(HEADS, HID).mean(axis=0).reshape(1, 16)
    biasm = np.ascontiguousarray(biasm, dtype=np.float32)

    # fT: [128, NFT*128] columns in table-build order (ft_node), zeros for pad
    ftn = plan.ft_node
    fT = np.zeros((IN_DIM, len(ftn)), dtype=bf16)
    real = ftn >= 0
    fT[:, np.nonzero(real)[0]] = features[ftn[real]].T.astype(bf16)
    fT = np.ascontiguousarray(fT)

    in_maps = []
    for c in range(plan.ncores):
        fLT = np.zeros((IN_DIM, plan.L), dtype=bf16)
        o = plan.order[c]
        real = o >= 0
        fLT[:, np.nonzero(real)[0]] = features[o[real]].T.astype(bf16)
        fLTB = np.zeros((IN_DIM, plan.L), dtype=bf16)
        ob = plan.orderB[c]
        realb = ob >= 0
        fLTB[:, np.nonzero(realb)[0]] = features[ob[realb]].T.astype(bf16)
        in_maps.append({
            "fT": fT,
            "fLT": np.ascontiguousarray(fLT),
            "fLTB": np.ascontiguousarray(fLTB),
            "Waug": Waug,
            "Wr": Wr_,
            "idx": np.ascontiguousarray(plan.idx_full[c]),
            "comb": np.ascontiguousarray(plan.comb_full[c]),
            "biasm": biasm,
        })
    return in_maps


def unshard_output(plan, outs):
    """outs: list of per-core {'out': [T*128,16]} -> full [N,16].
    Slot-major: node at orderB[c][t*128+p] lives at out row p*T + t."""
    res = np.empty((plan.N, 16), dtype=np.float32)
    T = plan.T
    for c in range(plan.ncores):
        o = plan.orderB[c]
        real = np.nonzero(o >= 0)[0]
        t = real // P
        p = real % P
        res[o[real]] = outs[c]["out"][p * T + t]
    return res


def kernel(features, W, attn_l, attn_r, bias, src, dst):
    from concourse.bass_utils import run_bass_kernel_spmd

    src = np.asarray(src)
    dst = np.asarray(dst)
    plan, nc = _get_plan_and_nc(src, dst, N_NODES, N_EDGES, NCORES)
    in_maps = make_inputs(plan, features, W, attn_l, attn_r, bias)
    res = run_bass_kernel_spmd(nc, in_maps, core_ids=list(range(NCORES)))
    return unshard_output(plan, res.results)


# revision 22
# speedup vs baseline: 1.4265x; 1.4265x over previous
"""GAT message-passing kernel for 8 Trainium2 NeuronCores.

Strategy (dst-sharded padded-CSR, no collectives):
  - Host: shard nodes into 8 contiguous ranges balanced by in-edge count.
    Edges follow their dst node; each core computes output rows for its own
    nodes only, so no cross-core reduction is needed.
  - Device, per core:
      Phase 1: project features -> per-node table rows [feat(64) | el(4) |
               junk] (256B rows) written to a DRAM table; el folded into the
               projection matmul via an augmented weight matrix. The host
               permutes fT columns so that matmul j emits, on partition p, the
               record of table row p*NT+j: each partition then holds a
               CONTIGUOUS run of table rows and the table write needs only
               128 large descriptors per slab (vs 1 per 256B row).
               A small second pass computes er for the core's own nodes.
      Phase 2: for each tile of 128 dst nodes, dma_gather the fat table rows
               of their (padded) src neighbor lists, compute
               ee = exp(leaky_relu(el_src + er_dst)) on DVE/ACT writing ee
               (bf16) back into the gathered rows' el slot, multiply feats by
               ee, and do ONE fused segment-reduce over [128, 68, C] that
               yields both the weighted feat sum and the ee sum.
               Softmax normalization is deferred: out = (sum ee*feat)/(sum ee).
  - int16 gather indices can't address 50000 rows, so the table is split at
    row 32768 (A = rows < 32768, B = rest) and each node's neighbor list is
    split into A/B sublists; nodes are tiled grouped by A-degree (B-pass uses
    an independent B-degree ordering plus an on-device combine gather).
    Dummy rows with el=-1e30 make padded slots contribute exactly zero.
"""

import os
import sys
from contextlib import ExitStack

import numpy as np

sys.path.insert(0, "/opt/trn_rl_repo")

# ---------------- problem constants (hardcoded per spec) ----------------
N_NODES = 50000
N_EDGES = 1600000
IN_DIM = 128
HEADS = 4
HID = 16
FEAT = HEADS * HID  # 64
NEG_SLOPE = 0.2
NCORES = 8
P = 128
ROW = 128          # table row size in bf16 elems (256 B)
EL_OFF = 64        # el lives at row[64:68] (f32 in bf16 elems 64:72)
NEG_BIG = -1.0e30

# table layout: [A nodes | A dummies][B nodes | B pad | B dummies]
A_CAP = 32768      # int16 addressing limit for the A side
NDUM = 512         # dummy rows per side
A_NODES = A_CAP - NDUM          # 32256 real nodes on the A side
NT_A = A_NODES // P             # 252 rows per partition chunk (A region)

# tunables
CMAX = int(os.environ.get("GAT_CMAX", "24"))   # max gather cols/unit
SLAB_TILES = int(os.environ.get("GAT_SLAB", "16"))  # node-tiles per p1 slab
NQUEUES = int(os.environ.get("GAT_NQUEUES", "4"))
DMA_SCRATCH = int(os.environ.get("GAT_SCRATCH", "16384"))
# a gather call of ck cols emits ~8*ck+1 tx descs and must fit one SDMA
# packet (<=64 data descs) => ck <= 8 per call with single_packet
CALLMAX = int(os.environ.get("GAT_CALLMAX", "8"))
CHAIN = os.environ.get("GAT_CHAIN", "1") == "1"
DEPTH = int(os.environ.get("GAT_DEPTH", "2"))  # in-flight gather calls/queue
GBUFS = int(os.environ.get("GAT_GBUFS", "8"))
SKIP_P2 = os.environ.get("GAT_SKIP_P2", "0") == "1"
SKIP_GATHER = os.environ.get("GAT_SKIP_GATHER", "0") == "1"
SKIP_COMPUTE = os.environ.get("GAT_SKIP_COMPUTE", "0") == "1"
NOMULT = os.environ.get("GAT_NOMULT", "0") == "1"      # timing probe
CREDUCE = os.environ.get("GAT_CREDUCE", "0") == "1"    # timing probe


def _round_up(x, m):
    return (x + m - 1) // m * m


# ---------------- host-side planning ----------------

class Plan:
    pass


def make_plan(src, dst, n_nodes, n_edges, ncores, cmax=CMAX):
    """Pure-index planning: core shards, node order, tile schedule, gather
    index arrays, table row mapping."""
    src = np.asarray(src).astype(np.int64)
    dst = np.asarray(dst).astype(np.int64)
    N = n_nodes
    E = n_edges

    deg = np.bincount(dst, minlength=N)
    cum = np.cumsum(deg)
    # node-id boundaries for the 8 cores, balanced by edge count
    targets = (np.arange(1, ncores) * E) // ncores
    inner = np.searchsorted(cum, targets, side="left") + 1
    bnds = np.concatenate([[0], inner, [N]]).astype(np.int64)
    Lc = np.diff(bnds)
    assert (Lc > 0).all()

    # ---- table row layout ----
    # A region: rows [0, A_NODES) in 128 partition-chunks of NT_A rows,
    #           A dummies [A_NODES, A_CAP)
    # B region: rows [A_CAP, A_CAP + 128*NT_B) (incl pad), B dummies after
    NB = N - A_NODES
    NT_B = _round_up(NB, P) // P          # rows per partition chunk (B)
    B_BASE = A_CAP
    B_ROWS = P * NT_B
    dummyB0 = B_BASE + B_ROWS
    TROWS = dummyB0 + NDUM
    assert B_ROWS + NDUM <= 32767, "B side exceeds int16 range"

    # node -> table row. Nodes 0..A_NODES-1 go to region A, rest to B.
    # Within a region, node k (0-based within region) -> partition-chunk
    # layout row: base + (k % P) * NT + (k // P). This makes the j-th p1
    # matmul (which processes nodes with k//P == j, k%P == p on partition p)
    # emit records that are contiguous-per-partition.
    nid = np.arange(N)
    k_a = nid  # region-local index for A nodes (only valid < A_NODES)
    rowA = (k_a % P) * NT_A + k_a // P
    k_b = nid - A_NODES
    rowB = B_BASE + (k_b % P) * NT_B + k_b // P
    tblrow = np.where(nid < A_NODES, rowA, rowB)
    # fT column order: column j*128+p must hold the node at region row
    # p*NT+j. Build inverse mapping: ft_node[col] = node id (or -1 for pad).
    ft_cols_A = np.full(P * NT_A, -1, dtype=np.int64)
    ft_cols_A[(k_a[:A_NODES] % P) * NT_A + k_a[:A_NODES] // P] = nid[:A_NODES]
    # stored column-major per matmul: col j*128+p <-> region row p*NT+j
    colsA = np.full((NT_A, P), -1, dtype=np.int64)
    colsA[k_a[:A_NODES] // P, k_a[:A_NODES] % P] = nid[:A_NODES]
    colsB = np.full((NT_B, P), -1, dtype=np.int64)
    colsB[k_b[A_NODES:] // P, k_b[A_NODES:] % P] = nid[A_NODES:]
    ft_node = np.concatenate([colsA.reshape(-1), colsB.reshape(-1)])

    isA = src < A_NODES
    a_deg = np.bincount(dst[isA], minlength=N)
    b_deg = deg - a_deg

    # per-core node ordering: A-pass tiles sorted by -a_deg, B-pass tiles
    # independently sorted by -b_deg; positions within core: slot-major
    T = int(max((Lc + P - 1) // P))
    L = T * P
    order = np.full((ncores, L), -1, dtype=np.int64)
    orderB = np.full((ncores, L), -1, dtype=np.int64)
    posl_a = np.empty(N, dtype=np.int64)  # within-core A-order position
    for c in range(ncores):
        ids = np.arange(bnds[c], bnds[c + 1])
        o = ids[np.lexsort((-b_deg[ids], -a_deg[ids]))]
        order[c, : len(o)] = o
        posl_a[o] = np.arange(len(o))
        ob = ids[np.lexsort((-a_deg[ids], -b_deg[ids]))]
        orderB[c, : len(ob)] = ob

    # edges sorted by (dst, then A-first)
    eorder = np.lexsort((~isA, dst))
    s_sorted = src[eorder]
    tid_sorted = tblrow[s_sorted]            # table row of each edge's src
    seg_start = np.concatenate([[0], cum])   # per-dst segment starts

    # per-(core,tile) max a/b degree -> global schedule (per-pass orders)
    KA = np.zeros((ncores, T), dtype=np.int64)
    KB = np.zeros((ncores, T), dtype=np.int64)
    for c in range(ncores):
        for t in range(T):
            sa = order[c, t * P : (t + 1) * P]
            sa = sa[sa >= 0]
            if len(sa):
                KA[c, t] = a_deg[sa].max()
            sb = orderB[c, t * P : (t + 1) * P]
            sb = sb[sb >= 0]
            if len(sb):
                KB[c, t] = b_deg[sb].max()
    KAg = KA.max(axis=0)
    KBg = KB.max(axis=0)

    # units: per tile-pass, chunks <= cmax cols, calls <= CALLMAX cols.
    # All A-units of all tiles run before any B-unit so the B-half of the
    # table build overlaps under the A gathers.
    def _side_units(t, side, k):
        out = []
        k0 = 0
        while k0 < k:
            cur = {"t": t, "side": side, "calls": [], "cols": 0}
            while cur["cols"] < cmax and k0 < k:
                ck = min(cmax - cur["cols"], k - k0, CALLMAX)
                cur["calls"].append((side, cur["cols"], k0, ck))
                cur["cols"] += ck
                k0 += ck
            out.append(cur)
        return out

    a_units, b_units = [], []
    for t in range(T):
        ua = _side_units(t, "A", int(KAg[t]))
        ub = _side_units(t, "B", int(KBg[t]))
        if not ua:
            ua = [{"t": t, "side": "A", "calls": [], "cols": 0}]
        if not ub:
            ub = [{"t": t, "side": "B", "calls": [], "cols": 0}]
        for u in ua + ub:
            u["first"] = False
            u["last"] = False
        ua[0]["first"] = True
        ua[-1]["alast"] = True
        ub[0]["first"] = True
        ub[-1]["last"] = True
        a_units.extend(ua)
        b_units.extend(ub)
    units = a_units + b_units

    # per-core int16 index arrays, packed per unit/call in 16-partition wrap
    totcols = 8 * sum(u["cols"] for u in units)
    totcols = max(totcols, 8)
    idx_arr = np.zeros((ncores, 16, totcols), dtype=np.int16)
    call_off = []
    off = 0
    for u in units:
        offs = []
        for side, gcol, k0, ck in u["calls"]:
            offs.append(off)
            off += 8 * ck
        call_off.append(offs)

    rngp = np.random.default_rng(12345)
    for c in range(ncores):
        colA_cache = {}
        colB_cache = {}
        for t in range(T):
            slots = order[c, t * P : (t + 1) * P]
            ok = slots >= 0
            sl = np.where(ok, slots, 0)
            al = np.where(ok, a_deg[sl], 0).astype(np.int64)
            st = seg_start[sl]
            slotsB = orderB[c, t * P : (t + 1) * P]
            okB = slotsB >= 0
            slB = np.where(okB, slotsB, 0)
            alB = np.where(okB, a_deg[slB], 0).astype(np.int64)
            bl = np.where(okB, b_deg[slB], 0).astype(np.int64)
            stB = seg_start[slB]
            ka, kb = int(KAg[t]), int(KBg[t])
            if ka > 0:
                arr = A_NODES + rngp.integers(0, NDUM, size=(P, ka))
                tot = int(al.sum())
                if tot:
                    cs = np.concatenate([[0], np.cumsum(al)[:-1]])
                    flat = np.repeat(st, al) + (np.arange(tot) - np.repeat(cs, al))
                    mask = np.arange(ka)[None, :] < al[:, None]
                    arr[mask] = tid_sorted[flat]
                colA_cache[t] = arr.T  # [ka, 128]
            if kb > 0:
                # B-relative rows: dummies at [dummyB0-B_BASE, +NDUM)
                arr = (dummyB0 - B_BASE) + rngp.integers(0, NDUM, size=(P, kb))
                tot = int(bl.sum())
                if tot:
                    cs = np.concatenate([[0], np.cumsum(bl)[:-1]])
                    flat = (np.repeat(stB + alB, bl)
                            + (np.arange(tot) - np.repeat(cs, bl)))
                    mask = np.arange(kb)[None, :] < bl[:, None]
                    arr[mask] = tid_sorted[flat] - B_BASE
                colB_cache[t] = arr.T
        for ui, u in enumerate(units):
            for (side, gcol, k0, ck), o in zip(u["calls"], call_off[ui]):
                cols = (colA_cache if side == "A" else colB_cache)[u["t"]]
                blk = cols[k0 : k0 + ck].reshape(-1)  # k-major flat, 128*ck
                assert blk.max() <= 32767 and blk.min() >= 0
                idx_arr[c, :, o : o + 8 * ck] = (
                    blk.reshape(-1, 16).T.astype(np.int16))

    idx_full = np.tile(idx_arr, (1, 8, 1))  # [ncores, 128, totcols]

    # combine gather: accA rows (within-core A-order) for each B slot,
    # packed k-major over tiles in chunks of 8 tiles (1024 idxs/call)
    CCH = 8
    ncomb = (T + CCH - 1) // CCH
    comb_cols = ncomb * CCH * 8
    comb_arr = np.zeros((ncores, 16, comb_cols), dtype=np.int16)
    for c in range(ncores):
        qa = np.zeros((T, P), dtype=np.int64)
        for t in range(T):
            sb = orderB[c, t * P : (t + 1) * P]
            okb = sb >= 0
            slot = posl_a[sb[okb]]
            # accAd rows are slot-major: row = (slot%128)*T + slot//128
            qa[t, okb] = (slot % P) * T + slot // P
        for j in range(ncomb):
            blk = qa[j * CCH : (j + 1) * CCH]
            if blk.shape[0] < CCH:
                blk = np.concatenate(
                    [blk, np.zeros((CCH - blk.shape[0], P), np.int64)])
            flat = blk.reshape(-1)
            comb_arr[c, :, j * CCH * 8 : (j + 1) * CCH * 8] = (
                flat.reshape(-1, 16).T.astype(np.int16))
    comb_full = np.tile(comb_arr, (1, 8, 1))

    p = Plan()
    p.N, p.E, p.ncores = N, E, ncores
    p.T, p.L, p.TROWS = T, L, TROWS
    p.NB, p.NT_B, p.B_BASE, p.B_ROWS = NB, NT_B, B_BASE, B_ROWS
    p.dummyB0 = dummyB0
    p.order, p.orderB, p.tblrow, p.ft_node = order, orderB, tblrow, ft_node
    p.comb_full, p.comb_cols, p.CCH = comb_full, comb_cols, CCH
    p.units, p.call_off, p.totcols = units, call_off, totcols
    p.idx_full = idx_full
    p.KAg, p.KBg = KAg, KBg
    return p


# ---------------- device kernel builder ----------------

def build_nc(plan, num_cores, slab_tiles=SLAB_TILES, cmax=CMAX, reps=1):
    import concourse.bacc as bacc
    import concourse.bass as bass
    import concourse.tile as tile
    from concourse import mybir
    from concourse.tile import add_dep_helper

    f32 = mybir.dt.float32
    bf16 = mybir.dt.bfloat16
    i16 = mybir.dt.int16
    Alu = mybir.AluOpType
    Act = mybir.ActivationFunctionType

    T, TROWS = plan.T, plan.TROWS
    totcols = plan.totcols
    NT_B = plan.NT_B
    NFT = NT_A + NT_B  # fT matmuls (table-build tiles)

    nc = bacc.Bacc("TRN2", target_bir_lowering=False, debug=False,
                   enable_asserts=False, num_devices=num_cores,
                   num_swdge_queues=NQUEUES,
                   dynamic_dma_scratch_size=DMA_SCRATCH)

    fT = nc.dram_tensor("fT", [P, NFT * P], bf16, kind="ExternalInput").ap()
    fLT = nc.dram_tensor("fLT", [P, T * P], bf16, kind="ExternalInput").ap()
    Waug = nc.dram_tensor("Waug", [P, 68], bf16, kind="ExternalInput").ap()
    Wr = nc.dram_tensor("Wr", [P, 4], bf16, kind="ExternalInput").ap()
    idx = nc.dram_tensor("idx", [P, totcols], i16, kind="ExternalInput").ap()
    comb = nc.dram_tensor("comb", [P, plan.comb_cols], i16,
                          kind="ExternalInput").ap()
    fLTB = nc.dram_tensor("fLTB", [P, T * P], bf16, kind="ExternalInput").ap()
    biasm = nc.dram_tensor("biasm", [1, 16], f32, kind="ExternalInput").ap()
    out = nc.dram_tensor("out", [T * P, 16], f32, kind="ExternalOutput").ap()

    with tile.TileContext(nc) as tc, ExitStack() as ctx:
        dpool = ctx.enter_context(tc.tile_pool(name="dram", bufs=1, space="DRAM"))
        wpool = ctx.enter_context(tc.tile_pool(name="wpool", bufs=1))
        nping = 2 if reps > 1 else 1
        tables = [dpool.tile([TROWS, ROW], bf16, name=f"table{b}")
                  for b in range(nping)]
        accAd = dpool.tile([T * P, ROW], bf16, name="accAd")
        erA_sbs = [wpool.tile([P, T, 4], f32, tag=f"erA{b}", name=f"erA{b}")
                   for b in range(nping)]
        erB_sbs = [wpool.tile([P, T, 4], f32, tag=f"erB{b}", name=f"erB{b}")
                   for b in range(nping)]
        accA_all = wpool.tile([P, T, 68], f32, tag="accA", name="accA")
        accB_all = wpool.tile([P, T, 68], f32, tag="accB", name="accB")
        accAbf = wpool.tile([P, T, ROW], bf16, tag="accAbf", name="accAbf")
        nc.vector.memset(accAbf[:], 0.0)
        ncomb = plan.comb_cols // 64
        G2 = wpool.tile([P, ncomb * plan.CCH, ROW], bf16, tag="G2", name="G2")
        outbuf = wpool.tile([P, T, 16], f32, tag="outbuf", name="outbuf")

        # static tiles, loaded once (weights/topology do not change per rep)
        waug_sb = wpool.tile([P, 68], bf16, tag="waug")
        wr_sb = wpool.tile([P, 4], bf16, tag="wr")
        biasm_sb = wpool.tile([P, 16], f32, tag="biasm")
        idx_sb = wpool.tile([P, totcols], i16, tag="idx")
        comb_sb = wpool.tile([P, plan.comb_cols], i16, tag="comb")
        nc.sync.dma_start(comb_sb[:], comb)
        nc.sync.dma_start(waug_sb[:], Waug)
        nc.sync.dma_start(wr_sb[:], Wr)
        nc.sync.dma_start(biasm_sb[:1, :], biasm)
        nc.gpsimd.partition_broadcast(biasm_sb[:], biasm_sb[:1, :])
        nc.sync.dma_start(idx_sb[:], idx)

        # dummy rows: el = -1e30 (feat junk is harmless: ee = 0 kills it).
        # Dummies are never overwritten by the per-rep table build, so init
        # them once. A dummies: rows [A_NODES, A_CAP); B: [dummyB0, TROWS).
        neg_sb = wpool.tile([P, NDUM // P, ROW], bf16, tag="neg")
        nc.vector.memset(neg_sb[:], 0.0)
        nc.vector.memset(
            neg_sb[:, :, EL_OFF : EL_OFF + 8].bitcast(f32), NEG_BIG)
        for b in range(nping):
            for d0 in (A_NODES, plan.dummyB0):
                nc.sync.dma_start(
                    tables[b][d0 : d0 + NDUM, :]
                    .rearrange("(j p) f -> p j f", p=P),
                    neg_sb[:])

        # transient pools shared by both ping-pong bodies
        slabp = ctx.enter_context(tc.tile_pool(name="slab", bufs=2))
        stagep = ctx.enter_context(tc.tile_pool(name="stage", bufs=2))
        psp = ctx.enter_context(tc.tile_pool(name="ps1", bufs=2, space="PSUM"))
        gp = ctx.enter_context(tc.tile_pool(name="gp", bufs=GBUFS))
        # ee lives in PSUM: 2-input DVE ops with one PSUM operand leave the
        # DVE/GpSimd shared SBUF port pair free, so SWDGE gather descriptor
        # generation isn't starved (see 01-sbuf.md SWDGE trap)
        eep = ctx.enter_context(tc.tile_pool(name="eep", bufs=3, space="PSUM"))
        esp = ctx.enter_context(tc.tile_pool(name="esp", bufs=3))
        tmpp = ctx.enter_context(tc.tile_pool(name="tmpp", bufs=6))

        qstate = {"qn": 0, "last": {}}

        def emit_p1(b):
            table = tables[b]

            # er for the core's own nodes (A order and B order)
            for src_t, er_dst in ((fLT, erA_sbs[b]), (fLTB, erB_sbs[b])):
                fl = slabp.tile([P, T * P], bf16, tag="fl")
                nc.sync.dma_start(fl[:], src_t)
                for t in range(T):
                    pse = psp.tile([P, 4], f32, tag="pse")
                    nc.tensor.matmul(out=pse[:],
                                     lhsT=fl[:, t * P : (t + 1) * P],
                                     rhs=wr_sb[:], start=True, stop=True)
                    nc.vector.tensor_copy(er_dst[:, t, :], pse[:])

            # table build: region A is fT tiles [0, NT_A), region B the rest.
            # fT column j*128+p holds the node of table row base + p*NT + j,
            # so partition p's stage row j is table row p*NT+j: the write is
            # one contiguous (nt*256B) descriptor per partition.
            def build_region(base_row, nt_region, ft_lo):
                t0 = 0
                while t0 < nt_region:
                    nt = min(slab_tiles, nt_region - t0)
                    slab = slabp.tile([P, slab_tiles * P], bf16, tag="slab")
                    nc.sync.dma_start(
                        slab[:, : nt * P],
                        fT[:, (ft_lo + t0) * P : (ft_lo + t0 + nt) * P])
                    stage = stagep.tile([P, slab_tiles, ROW], bf16,
                                        tag="stage")
                    for j in range(nt):
                        ps = psp.tile([P, 68], f32, tag="ps")
                        nc.tensor.matmul(out=ps[:],
                                         lhsT=slab[:, j * P : (j + 1) * P],
                                         rhs=waug_sb[:], start=True, stop=True)
                        nc.scalar.activation(stage[:, j, 0:64], ps[:, 0:64],
                                             Act.Copy)
                        nc.scalar.activation(
                            stage[:, j, 64:72].bitcast(f32), ps[:, 64:68],
                            Act.Copy)
                    # partition p -> rows [base + p*nt_region + t0, +nt);
                    # writes ride the ACT HWDGE ring so they don't block the
                    # next slab load (SP ring) behind their stage dependency
                    dst = table[base_row : base_row + P * nt_region, :] \
                        .rearrange("(p j) f -> p j f", p=P)
                    nc.scalar.dma_start(
                        dst[:, t0 : t0 + nt, :], stage[:, :nt, :])
                    t0 += nt

            build_region(0, NT_A, 0)
            build_region(plan.B_BASE, NT_B, NT_A)

        def emit_p2(b):
            table = tables[b]
            tabA = table[0:A_CAP, :]
            tabB = table[plan.B_BASE : TROWS, :]
            flushed = [False]
            for ui, u in enumerate(plan.units):
                if SKIP_P2:
                    break
                t = u["t"]
                C = u["cols"]
                isA_u = u["side"] == "A"
                if not isA_u and not flushed[0]:
                    # end of A pass: flush all A accumulators (cast f32->bf16)
                    # to DRAM in one contiguous slot-major DMA for the combine
                    flushed[0] = True
                    if not (SKIP_GATHER or SKIP_COMPUTE):
                        nc.vector.tensor_copy(accAbf[:, :, 0:68], accA_all[:])
                    nc.scalar.dma_start(
                        accAd.rearrange("(p t) f -> p t f", p=P), accAbf[:])
                if not isA_u and u["first"] and t % plan.CCH == 0:
                    # A accumulators (bf16 rows in accAd) -> B slot order
                    j = t // plan.CCH
                    gi = nc.gpsimd.dma_gather(
                        G2[:, j * plan.CCH : (j + 1) * plan.CCH, :],
                        accAd,
                        comb_sb[:, j * 8 * plan.CCH : (j + 1) * 8 * plan.CCH],
                        plan.CCH * P, plan.CCH * P, ROW,
                        queue_num=qstate["qn"])
                    if CHAIN:
                        hist = qstate["last"].setdefault(qstate["qn"], [])
                        hist.append(gi.ins)
                        if len(hist) > DEPTH:
                            add_dep_helper(gi.ins, hist.pop(0),
                                           reason="swdge ring throttle")
                    qstate["qn"] = (qstate["qn"] + 1) % NQUEUES
                er_sb = erA_sbs[b] if isA_u else erB_sbs[b]
                acc_all = accA_all if isA_u else accB_all
                acc68 = acc_all[:, t, 0:68]

                def do_gathers(G):
                    for (side, gcol, k0, ck), o in zip(u["calls"],
                                                       plan.call_off[ui]):
                        tab = tabA if side == "A" else tabB
                        gi = nc.gpsimd.dma_gather(
                            G[:, gcol : gcol + ck, :], tab,
                            idx_sb[:, o : o + 8 * ck],
                            ck * P, ck * P, ROW, queue_num=qstate["qn"])
                        if CHAIN:
                            hist = qstate["last"].setdefault(qstate["qn"], [])
                            hist.append(gi.ins)
                            if len(hist) > DEPTH:
                                add_dep_helper(gi.ins, hist.pop(0),
                                               reason="swdge ring throttle")
                        qstate["qn"] = (qstate["qn"] + 1) % NQUEUES

                if SKIP_GATHER or SKIP_COMPUTE:
                    G = gp.tile([P, cmax, ROW], bf16, tag="G")
                    if not SKIP_GATHER:
                        do_gathers(G)
                    if u["last"] and not isA_u:
                        fin = tmpp.tile([P, 16], f32, tag="fin")
                        if SKIP_GATHER:
                            nc.vector.memset(fin[:], 0.0)
                        else:
                            nc.vector.tensor_reduce(
                                fin[:], G[:, :max(C, 1), 0:16]
                                .rearrange("p c f -> p f c"),
                                mybir.AxisListType.X, Alu.add)
                        nc.vector.tensor_copy(outbuf[:, t, :], fin[:])
                    continue
                if C > 0:
                    G = gp.tile([P, cmax, ROW], bf16, tag="G")
                    do_gathers(G)
                    es = esp.tile([P, cmax, 4], f32, tag="es")
                    ee = eep.tile([P, cmax, 4], f32, tag="ee")
                    # e = el_src + er_dst, leaky-relu (small SBUF ops)
                    nc.vector.tensor_tensor(
                        es[:, :C, :],
                        G[:, :C, EL_OFF : EL_OFF + 8].bitcast(f32),
                        er_sb[:, t : t + 1, :].to_broadcast([P, C, 4]),
                        Alu.add)
                    nc.vector.scalar_tensor_tensor(
                        es[:, :C, :], es[:, :C, :], NEG_SLOPE, es[:, :C, :],
                        Alu.mult, Alu.max)
                    # exp lands in PSUM so the big mult's second operand
                    # stays off the shared SBUF port pair
                    nc.scalar.activation(ee[:, :C, :], es[:, :C, :], Act.Exp)
                    # msg = feat * ee (ee broadcast from PSUM: keeps the
                    # shared SBUF port free for SWDGE descriptor writes)
                    if not NOMULT:
                        nc.vector.tensor_tensor(
                            G[:, :C, 0:FEAT].rearrange("p c (h d) -> p c h d",
                                                       h=HEADS),
                            G[:, :C, 0:FEAT].rearrange("p c (h d) -> p c h d",
                                                       h=HEADS),
                            ee[:, :C, :].unsqueeze(-1)
                            .to_broadcast([P, C, HEADS, HID]),
                            Alu.mult)
                    # k-reductions: msg from SBUF (1-input), ee from PSUM
                    msum = G[:, :C, 0:FEAT].rearrange("p c f -> p f c")
                    dsum = ee[:, :C, :].rearrange("p c h -> p h c")
                    if u["first"]:
                        nc.vector.tensor_reduce(
                            acc68[:, 0:FEAT], msum, mybir.AxisListType.X,
                            Alu.add)
                        nc.vector.tensor_reduce(
                            acc68[:, FEAT : FEAT + 4], dsum,
                            mybir.AxisListType.X, Alu.add)
                    else:
                        tmp = tmpp.tile([P, 68], f32, tag="tmp")
                        nc.vector.tensor_reduce(
                            tmp[:, 0:FEAT], msum, mybir.AxisListType.X,
                            Alu.add)
                        nc.vector.tensor_reduce(
                            tmp[:, FEAT : FEAT + 4], dsum,
                            mybir.AxisListType.X, Alu.add)
                        nc.vector.tensor_add(acc68, acc68, tmp[:])
                elif u["first"]:
                    nc.vector.memset(acc68, 0.0)
                if u["last"] and not isA_u:
                    tmp68 = tmpp.tile([P, 68], f32, tag="tmp68")
                    nc.vector.tensor_tensor(
                        tmp68[:], G2[:, t, 0:68], acc68, Alu.add)
                    dent = tmp68[:, FEAT : FEAT + 4]
                    nc.vector.tensor_scalar_max(dent, dent, 1e-30)
                    rec = tmpp.tile([P, 4], f32, tag="rec")
                    nc.vector.reciprocal(rec[:], dent)
                    nc.vector.tensor_tensor(
                        tmp68[:, 0:FEAT].rearrange("p (h d) -> p h d",
                                                   h=HEADS),
                        tmp68[:, 0:FEAT].rearrange("p (h d) -> p h d",
                                                   h=HEADS),
                        rec[:].unsqueeze(-1).to_broadcast([P, HEADS, HID]),
                        Alu.mult)
                    fin = tmpp.tile([P, 16], f32, tag="fin")
                    nc.vector.tensor_reduce(
                        fin[:],
                        tmp68[:, 0:FEAT].rearrange("p (h d) -> p d h",
                                                   h=HEADS),
                        mybir.AxisListType.X, Alu.add)
                    nc.vector.scalar_tensor_tensor(
                        outbuf[:, t, :], fin[:], 1.0 / HEADS, biasm_sb[:],
                        Alu.mult, Alu.add)
            if not SKIP_P2:
                # slot-major output: out row p*T + t <- outbuf[p, t, :]
                nc.sync.dma_start(
                    out.rearrange("(p t) f -> p t f", p=P), outbuf[:])

        if reps > 1:
            R = reps - (reps % 2)
            emit_p1(0)
            if R // 2 - 1 > 0:
                with tc.For_i(0, R // 2 - 1, 1):
                    emit_p1(1)
                    emit_p2(0)
                    emit_p1(0)
                    emit_p2(1)
            emit_p1(1)
            emit_p2(0)
            emit_p2(1)
            if reps % 2:
                emit_p1(0)
                emit_p2(0)
        else:
            emit_p1(0)
            emit_p2(0)

    nc.compile()
    return nc


# ---------------- host wrapper ----------------

_CACHE = {}


def _get_plan_and_nc(src, dst, n_nodes, n_edges, ncores, cmax=CMAX,
                     slab_tiles=SLAB_TILES):
    key = (int(src[0]), int(src[-1]), int(dst[0]), int(dst[-1]),
           len(src), n_nodes, ncores, cmax)
    if key not in _CACHE:
        plan = make_plan(src, dst, n_nodes, n_edges, ncores, cmax=cmax)
        nc = build_nc(plan, ncores, slab_tiles=slab_tiles, cmax=cmax)
        _CACHE[key] = (plan, nc)
    return _CACHE[key]


def make_inputs(plan, features, W, attn_l, attn_r, bias):
    """Build per-core input maps from full inputs + plan."""
    import ml_dtypes

    bf16 = ml_dtypes.bfloat16
    features = np.asarray(features, dtype=np.float32)
    W = np.asarray(W, dtype=np.float32)
    attn_l = np.asarray(attn_l, dtype=np.float32)
    attn_r = np.asarray(attn_r, dtype=np.float32)
    bias = np.asarray(bias, dtype=np.float32)

    # augmented weights
    W3 = W.reshape(IN_DIM, HEADS, HID)
    Wl = np.einsum("ihd,hd->ih", W3, attn_l).astype(np.float32)
    Wr_ = np.einsum("ihd,hd->ih", W3, attn_r).astype(bf16)
    Waug = np.concatenate([W, Wl], axis=1).astype(bf16)  # [128, 68]
    biasm = bias.reshape(HEADS, HID).mean(axis=0).reshape(1, 16)
    biasm = np.ascontiguousarray(biasm, dtype=np.float32)

    # fT: [128, NFT*128] columns in table-build order (ft_node), zeros for pad
    ftn = plan.ft_node
    fT = np.zeros((IN_DIM, len(ftn)), dtype=bf16)
    real = ftn >= 0
    fT[:, np.nonzero(real)[0]] = features[ftn[real]].T.astype(bf16)
    fT = np.ascontiguousarray(fT)

    in_maps = []
    for c in range(plan.ncores):
        fLT = np.zeros((IN_DIM, plan.L), dtype=bf16)
        o = plan.order[c]
        real = o >= 0
        fLT[:, np.nonzero(real)[0]] = features[o[real]].T.astype(bf16)
        fLTB = np.zeros((IN_DIM, plan.L), dtype=bf16)
        ob = plan.orderB[c]
        realb = ob >= 0
        fLTB[:, np.nonzero(realb)[0]] = features[ob[realb]].T.astype(bf16)
        in_maps.append({
            "fT": fT,
            "fLT": np.ascontiguousarray(fLT),
            "fLTB": np.ascontiguousarray(fLTB),
            "Waug": Waug,
            "Wr": Wr_,
            "idx": np.ascontiguousarray(plan.idx_full[c]),
            "comb": np.ascontiguousarray(plan.comb_full[c]),
            "biasm": biasm,
        })
    return in_maps


def unshard_output(plan, outs):
    """outs: list of per-core {'out': [T*128,16]} -> full [N,16].
    Slot-major: node at orderB[c][t*128+p] lives at out row p*T + t."""
    res = np.empty((plan.N, 16), dtype=np.float32)
    T = plan.T
    for c in range(plan.ncores):
        o = plan.orderB[c]
        real = np.nonzero(o >= 0)[0]
        t = real // P
        p = real % P
        res[o[real]] = outs[c]["out"][p * T + t]
    return res


def kernel(features, W, attn_l, attn_r, bias, src, dst):
    from concourse.bass_utils import run_bass_kernel_spmd

    src = np.asarray(src)
    dst = np.asarray(dst)
    plan, nc = _get_plan_and_nc(src, dst, N_NODES, N_EDGES, NCORES)
    in_maps = make_inputs(plan, features, W, attn_l, attn_r, bias)
    res = run_bass_kernel_spmd(nc, in_maps, core_ids=list(range(NCORES)))
    return unshard_output(plan, res.results)


# revision 25
# speedup vs baseline: 1.4737x; 1.0330x over previous
"""GAT message-passing kernel for 8 Trainium2 NeuronCores.

Strategy (dst-sharded padded-CSR, no collectives):
  - Host: shard nodes into 8 contiguous ranges balanced by in-edge count.
    Edges follow their dst node; each core computes output rows for its own
    nodes only, so no cross-core reduction is needed.
  - Device, per core:
      Phase 1: project features -> per-node table rows [feat(64) | el(4) |
               junk] (256B rows) written to a DRAM table; el folded into the
               projection matmul via an augmented weight matrix. The host
               permutes fT columns so that matmul j emits, on partition p, the
               record of table row p*NT+j: each partition then holds a
               CONTIGUOUS run of table rows and the table write needs only
               128 large descriptors per slab (vs 1 per 256B row).
               A small second pass computes er for the core's own nodes.
      Phase 2: for each tile of 128 dst nodes, dma_gather the fat table rows
               of their (padded) src neighbor lists, compute
               ee = exp(leaky_relu(el_src + er_dst)) on DVE/ACT writing ee
               (bf16) back into the gathered rows' el slot, multiply feats by
               ee, and do ONE fused segment-reduce over [128, 68, C] that
               yields both the weighted feat sum and the ee sum.
               Softmax normalization is deferred: out = (sum ee*feat)/(sum ee).
  - int16 gather indices can't address 50000 rows, so the table is split at
    row 32768 (A = rows < 32768, B = rest) and each node's neighbor list is
    split into A/B sublists; nodes are tiled grouped by A-degree (B-pass uses
    an independent B-degree ordering plus an on-device combine gather).
    Dummy rows with el=-1e30 make padded slots contribute exactly zero.
"""

import os
import sys
from contextlib import ExitStack

import numpy as np

sys.path.insert(0, "/opt/trn_rl_repo")

# ---------------- problem constants (hardcoded per spec) ----------------
N_NODES = 50000
N_EDGES = 1600000
IN_DIM = 128
HEADS = 4
HID = 16
FEAT = HEADS * HID  # 64
NEG_SLOPE = 0.2
NCORES = 8
P = 128
ROW = 128          # table row size in bf16 elems (256 B)
EL_OFF = 64        # el lives at row[64:68] (f32 in bf16 elems 64:72)
NEG_BIG = -1.0e30

# table layout: [A nodes | A dummies][B nodes | B pad | B dummies]
A_CAP = 32768      # int16 addressing limit for the A side
NDUM = 512         # dummy rows per side
A_NODES = A_CAP - NDUM          # 32256 real nodes on the A side
NT_A = A_NODES // P             # 252 rows per partition chunk (A region)

# tunables
CMAX = int(os.environ.get("GAT_CMAX", "16"))   # max gather cols/unit
SLAB_TILES = int(os.environ.get("GAT_SLAB", "16"))  # node-tiles per p1 slab
NQUEUES = int(os.environ.get("GAT_NQUEUES", "4"))
DMA_SCRATCH = int(os.environ.get("GAT_SCRATCH", "32768"))
# a gather call of ck cols emits ~8*ck+1 tx descs and must fit one SDMA
# packet (<=64 data descs) => ck <= 8 per call with single_packet
CALLMAX = int(os.environ.get("GAT_CALLMAX", "8"))
CHAIN = os.environ.get("GAT_CHAIN", "1") == "1"
DEPTH = int(os.environ.get("GAT_DEPTH", "3"))  # in-flight gather calls/queue
GBUFS = int(os.environ.get("GAT_GBUFS", "10"))
SKIP_P2 = os.environ.get("GAT_SKIP_P2", "0") == "1"
SKIP_GATHER = os.environ.get("GAT_SKIP_GATHER", "0") == "1"
SKIP_COMPUTE = os.environ.get("GAT_SKIP_COMPUTE", "0") == "1"
NOMULT = os.environ.get("GAT_NOMULT", "0") == "1"      # timing probe
CREDUCE = os.environ.get("GAT_CREDUCE", "0") == "1"    # timing probe


def _round_up(x, m):
    return (x + m - 1) // m * m


# ---------------- host-side planning ----------------

class Plan:
    pass


def make_plan(src, dst, n_nodes, n_edges, ncores, cmax=CMAX):
    """Pure-index planning: core shards, node order, tile schedule, gather
    index arrays, table row mapping."""
    src = np.asarray(src).astype(np.int64)
    dst = np.asarray(dst).astype(np.int64)
    N = n_nodes
    E = n_edges

    deg = np.bincount(dst, minlength=N)
    cum = np.cumsum(deg)
    # node-id boundaries for the 8 cores, balanced by edge count
    targets = (np.arange(1, ncores) * E) // ncores
    inner = np.searchsorted(cum, targets, side="left") + 1
    bnds = np.concatenate([[0], inner, [N]]).astype(np.int64)
    Lc = np.diff(bnds)
    assert (Lc > 0).all()

    # ---- table row layout ----
    # A region: rows [0, A_NODES) in 128 partition-chunks of NT_A rows,
    #           A dummies [A_NODES, A_CAP)
    # B region: rows [A_CAP, A_CAP + 128*NT_B) (incl pad), B dummies after
    NB = N - A_NODES
    NT_B = _round_up(NB, P) // P          # rows per partition chunk (B)
    B_BASE = A_CAP
    B_ROWS = P * NT_B
    dummyB0 = B_BASE + B_ROWS
    TROWS = dummyB0 + NDUM
    assert B_ROWS + NDUM <= 32767, "B side exceeds int16 range"

    # node -> table row. Nodes 0..A_NODES-1 go to region A, rest to B.
    # Within a region, node k (0-based within region) -> partition-chunk
    # layout row: base + (k % P) * NT + (k // P). This makes the j-th p1
    # matmul (which processes nodes with k//P == j, k%P == p on partition p)
    # emit records that are contiguous-per-partition.
    nid = np.arange(N)
    k_a = nid  # region-local index for A nodes (only valid < A_NODES)
    rowA = (k_a % P) * NT_A + k_a // P
    k_b = nid - A_NODES
    rowB = B_BASE + (k_b % P) * NT_B + k_b // P
    tblrow = np.where(nid < A_NODES, rowA, rowB)
    # fT column order: column j*128+p must hold the node at region row
    # p*NT+j. Build inverse mapping: ft_node[col] = node id (or -1 for pad).
    ft_cols_A = np.full(P * NT_A, -1, dtype=np.int64)
    ft_cols_A[(k_a[:A_NODES] % P) * NT_A + k_a[:A_NODES] // P] = nid[:A_NODES]
    # stored column-major per matmul: col j*128+p <-> region row p*NT+j
    colsA = np.full((NT_A, P), -1, dtype=np.int64)
    colsA[k_a[:A_NODES] // P, k_a[:A_NODES] % P] = nid[:A_NODES]
    colsB = np.full((NT_B, P), -1, dtype=np.int64)
    colsB[k_b[A_NODES:] // P, k_b[A_NODES:] % P] = nid[A_NODES:]
    ft_node = np.concatenate([colsA.reshape(-1), colsB.reshape(-1)])

    isA = src < A_NODES
    a_deg = np.bincount(dst[isA], minlength=N)
    b_deg = deg - a_deg

    # per-core node ordering: A-pass tiles sorted by -a_deg, B-pass tiles
    # independently sorted by -b_deg; positions within core: slot-major
    T = int(max((Lc + P - 1) // P))
    L = T * P
    order = np.full((ncores, L), -1, dtype=np.int64)
    orderB = np.full((ncores, L), -1, dtype=np.int64)
    posl_a = np.empty(N, dtype=np.int64)  # within-core A-order position
    for c in range(ncores):
        ids = np.arange(bnds[c], bnds[c + 1])
        o = ids[np.lexsort((-b_deg[ids], -a_deg[ids]))]
        order[c, : len(o)] = o
        posl_a[o] = np.arange(len(o))
        ob = ids[np.lexsort((-a_deg[ids], -b_deg[ids]))]
        orderB[c, : len(ob)] = ob

    # edges sorted by (dst, then A-first)
    eorder = np.lexsort((~isA, dst))
    s_sorted = src[eorder]
    tid_sorted = tblrow[s_sorted]            # table row of each edge's src
    seg_start = np.concatenate([[0], cum])   # per-dst segment starts

    # per-(core,tile) max a/b degree -> global schedule (per-pass orders)
    KA = np.zeros((ncores, T), dtype=np.int64)
    KB = np.zeros((ncores, T), dtype=np.int64)
    for c in range(ncores):
        for t in range(T):
            sa = order[c, t * P : (t + 1) * P]
            sa = sa[sa >= 0]
            if len(sa):
                KA[c, t] = a_deg[sa].max()
            sb = orderB[c, t * P : (t + 1) * P]
            sb = sb[sb >= 0]
            if len(sb):
                KB[c, t] = b_deg[sb].max()
    KAg = KA.max(axis=0)
    KBg = KB.max(axis=0)

    # units: per tile-pass, chunks <= cmax cols, calls <= CALLMAX cols.
    # All A-units of all tiles run before any B-unit so the B-half of the
    # table build overlaps under the A gathers.
    def _side_units(t, side, k):
        out = []
        k0 = 0
        while k0 < k:
            cur = {"t": t, "side": side, "calls": [], "cols": 0}
            while cur["cols"] < cmax and k0 < k:
                ck = min(cmax - cur["cols"], k - k0, CALLMAX)
                cur["calls"].append((side, cur["cols"], k0, ck))
                cur["cols"] += ck
                k0 += ck
            out.append(cur)
        return out

    a_units, b_units = [], []
    for t in range(T):
        ua = _side_units(t, "A", int(KAg[t]))
        ub = _side_units(t, "B", int(KBg[t]))
        if not ua:
            ua = [{"t": t, "side": "A", "calls": [], "cols": 0}]
        if not ub:
            ub = [{"t": t, "side": "B", "calls": [], "cols": 0}]
        for u in ua + ub:
            u["first"] = False
            u["last"] = False
        ua[0]["first"] = True
        ua[-1]["alast"] = True
        ub[0]["first"] = True
        ub[-1]["last"] = True
        a_units.extend(ua)
        b_units.extend(ub)
    units = a_units + b_units

    # per-core int16 index arrays, packed per unit/call in 16-partition wrap
    totcols = 8 * sum(u["cols"] for u in units)
    totcols = max(totcols, 8)
    idx_arr = np.zeros((ncores, 16, totcols), dtype=np.int16)
    call_off = []
    off = 0
    for u in units:
        offs = []
        for side, gcol, k0, ck in u["calls"]:
            offs.append(off)
            off += 8 * ck
        call_off.append(offs)

    rngp = np.random.default_rng(12345)
    for c in range(ncores):
        colA_cache = {}
        colB_cache = {}
        for t in range(T):
            slots = order[c, t * P : (t + 1) * P]
            ok = slots >= 0
            sl = np.where(ok, slots, 0)
            al = np.where(ok, a_deg[sl], 0).astype(np.int64)
            st = seg_start[sl]
            slotsB = orderB[c, t * P : (t + 1) * P]
            okB = slotsB >= 0
            slB = np.where(okB, slotsB, 0)
            alB = np.where(okB, a_deg[slB], 0).astype(np.int64)
            bl = np.where(okB, b_deg[slB], 0).astype(np.int64)
            stB = seg_start[slB]
            ka, kb = int(KAg[t]), int(KBg[t])
            if ka > 0:
                arr = A_NODES + rngp.integers(0, NDUM, size=(P, ka))
                tot = int(al.sum())
                if tot:
                    cs = np.concatenate([[0], np.cumsum(al)[:-1]])
                    flat = np.repeat(st, al) + (np.arange(tot) - np.repeat(cs, al))
                    mask = np.arange(ka)[None, :] < al[:, None]
                    arr[mask] = tid_sorted[flat]
                colA_cache[t] = arr.T  # [ka, 128]
            if kb > 0:
                # B-relative rows: dummies at [dummyB0-B_BASE, +NDUM)
                arr = (dummyB0 - B_BASE) + rngp.integers(0, NDUM, size=(P, kb))
                tot = int(bl.sum())
                if tot:
                    cs = np.concatenate([[0], np.cumsum(bl)[:-1]])
                    flat = (np.repeat(stB + alB, bl)
                            + (np.arange(tot) - np.repeat(cs, bl)))
                    mask = np.arange(kb)[None, :] < bl[:, None]
                    arr[mask] = tid_sorted[flat] - B_BASE
                colB_cache[t] = arr.T
        for ui, u in enumerate(units):
            for (side, gcol, k0, ck), o in zip(u["calls"], call_off[ui]):
                cols = (colA_cache if side == "A" else colB_cache)[u["t"]]
                blk = cols[k0 : k0 + ck].reshape(-1)  # k-major flat, 128*ck
                assert blk.max() <= 32767 and blk.min() >= 0
                idx_arr[c, :, o : o + 8 * ck] = (
                    blk.reshape(-1, 16).T.astype(np.int16))

    idx_full = np.tile(idx_arr, (1, 8, 1))  # [ncores, 128, totcols]

    # combine gather: accA rows (within-core A-order) for each B slot,
    # packed k-major over tiles in chunks of 8 tiles (1024 idxs/call)
    CCH = 8
    ncomb = (T + CCH - 1) // CCH
    comb_cols = ncomb * CCH * 8
    comb_arr = np.zeros((ncores, 16, comb_cols), dtype=np.int16)
    for c in range(ncores):
        qa = np.zeros((T, P), dtype=np.int64)
        for t in range(T):
            sb = orderB[c, t * P : (t + 1) * P]
            okb = sb >= 0
            slot = posl_a[sb[okb]]
            # accAd rows are slot-major: row = (slot%128)*T + slot//128
            qa[t, okb] = (slot % P) * T + slot // P
        for j in range(ncomb):
            blk = qa[j * CCH : (j + 1) * CCH]
            if blk.shape[0] < CCH:
                blk = np.concatenate(
                    [blk, np.zeros((CCH - blk.shape[0], P), np.int64)])
            flat = blk.reshape(-1)
            comb_arr[c, :, j * CCH * 8 : (j + 1) * CCH * 8] = (
                flat.reshape(-1, 16).T.astype(np.int16))
    comb_full = np.tile(comb_arr, (1, 8, 1))

    p = Plan()
    p.N, p.E, p.ncores = N, E, ncores
    p.T, p.L, p.TROWS = T, L, TROWS
    p.NB, p.NT_B, p.B_BASE, p.B_ROWS = NB, NT_B, B_BASE, B_ROWS
    p.dummyB0 = dummyB0
    p.order, p.orderB, p.tblrow, p.ft_node = order, orderB, tblrow, ft_node
    p.comb_full, p.comb_cols, p.CCH = comb_full, comb_cols, CCH
    p.units, p.call_off, p.totcols = units, call_off, totcols
    p.idx_full = idx_full
    p.KAg, p.KBg = KAg, KBg
    return p


# ---------------- device kernel builder ----------------

def build_nc(plan, num_cores, slab_tiles=SLAB_TILES, cmax=CMAX, reps=1):
    import concourse.bacc as bacc
    import concourse.bass as bass
    import concourse.tile as tile
    from concourse import mybir
    from concourse.tile import add_dep_helper

    f32 = mybir.dt.float32
    bf16 = mybir.dt.bfloat16
    i16 = mybir.dt.int16
    Alu = mybir.AluOpType
    Act = mybir.ActivationFunctionType

    T, TROWS = plan.T, plan.TROWS
    totcols = plan.totcols
    NT_B = plan.NT_B
    NFT = NT_A + NT_B  # fT matmuls (table-build tiles)

    nc = bacc.Bacc("TRN2", target_bir_lowering=False, debug=False,
                   enable_asserts=False, num_devices=num_cores,
                   num_swdge_queues=NQUEUES,
                   dynamic_dma_scratch_size=DMA_SCRATCH)

    fT = nc.dram_tensor("fT", [P, NFT * P], bf16, kind="ExternalInput").ap()
    fLT = nc.dram_tensor("fLT", [P, T * P], bf16, kind="ExternalInput").ap()
    Waug = nc.dram_tensor("Waug", [P, 68], bf16, kind="ExternalInput").ap()
    Wr = nc.dram_tensor("Wr", [P, 4], bf16, kind="ExternalInput").ap()
    idx = nc.dram_tensor("idx", [P, totcols], i16, kind="ExternalInput").ap()
    comb = nc.dram_tensor("comb", [P, plan.comb_cols], i16,
                          kind="ExternalInput").ap()
    fLTB = nc.dram_tensor("fLTB", [P, T * P], bf16, kind="ExternalInput").ap()
    biasm = nc.dram_tensor("biasm", [1, 16], f32, kind="ExternalInput").ap()
    out = nc.dram_tensor("out", [T * P, 16], f32, kind="ExternalOutput").ap()

    with tile.TileContext(nc) as tc, ExitStack() as ctx:
        dpool = ctx.enter_context(tc.tile_pool(name="dram", bufs=1, space="DRAM"))
        wpool = ctx.enter_context(tc.tile_pool(name="wpool", bufs=1))
        nping = 2 if reps > 1 else 1
        tables = [dpool.tile([TROWS, ROW], bf16, name=f"table{b}")
                  for b in range(nping)]
        accAd = dpool.tile([T * P, ROW], bf16, name="accAd")
        erA_sbs = [wpool.tile([P, T, 4], f32, tag=f"erA{b}", name=f"erA{b}")
                   for b in range(nping)]
        erB_sbs = [wpool.tile([P, T, 4], f32, tag=f"erB{b}", name=f"erB{b}")
                   for b in range(nping)]
        accA_all = wpool.tile([P, T, 68], f32, tag="accA", name="accA")
        accB_all = wpool.tile([P, T, 68], f32, tag="accB", name="accB")
        accAbf = wpool.tile([P, T, ROW], bf16, tag="accAbf", name="accAbf")
        nc.vector.memset(accAbf[:], 0.0)
        ncomb = plan.comb_cols // 64
        G2 = wpool.tile([P, ncomb * plan.CCH, ROW], bf16, tag="G2", name="G2")
        outbuf = wpool.tile([P, T, 16], f32, tag="outbuf", name="outbuf")

        # static tiles, loaded once (weights/topology do not change per rep)
        waug_sb = wpool.tile([P, 68], bf16, tag="waug")
        wr_sb = wpool.tile([P, 4], bf16, tag="wr")
        biasm_sb = wpool.tile([P, 16], f32, tag="biasm")
        idx_sb = wpool.tile([P, totcols], i16, tag="idx")
        comb_sb = wpool.tile([P, plan.comb_cols], i16, tag="comb")
        nc.sync.dma_start(comb_sb[:], comb)
        nc.sync.dma_start(waug_sb[:], Waug)
        nc.sync.dma_start(wr_sb[:], Wr)
        nc.sync.dma_start(biasm_sb[:1, :], biasm)
        nc.gpsimd.partition_broadcast(biasm_sb[:], biasm_sb[:1, :])
        nc.sync.dma_start(idx_sb[:], idx)

        # dummy rows: el = -1e30 (feat junk is harmless: ee = 0 kills it).
        # Dummies are never overwritten by the per-rep table build, so init
        # them once. A dummies: rows [A_NODES, A_CAP); B: [dummyB0, TROWS).
        neg_sb = wpool.tile([P, NDUM // P, ROW], bf16, tag="neg")
        nc.vector.memset(neg_sb[:], 0.0)
        nc.vector.memset(
            neg_sb[:, :, EL_OFF : EL_OFF + 8].bitcast(f32), NEG_BIG)
        for b in range(nping):
            for d0 in (A_NODES, plan.dummyB0):
                nc.sync.dma_start(
                    tables[b][d0 : d0 + NDUM, :]
                    .rearrange("(j p) f -> p j f", p=P),
                    neg_sb[:])

        # transient pools shared by both ping-pong bodies
        slabp = ctx.enter_context(tc.tile_pool(name="slab", bufs=2))
        stagep = ctx.enter_context(tc.tile_pool(name="stage", bufs=2))
        psp = ctx.enter_context(tc.tile_pool(name="ps1", bufs=2, space="PSUM"))
        gp = ctx.enter_context(tc.tile_pool(name="gp", bufs=GBUFS))
        # ee lives in PSUM: 2-input DVE ops with one PSUM operand leave the
        # DVE/GpSimd shared SBUF port pair free, so SWDGE gather descriptor
        # generation isn't starved (see 01-sbuf.md SWDGE trap)
        eep = ctx.enter_context(tc.tile_pool(name="eep", bufs=3, space="PSUM"))
        esp = ctx.enter_context(tc.tile_pool(name="esp", bufs=3))
        tmpp = ctx.enter_context(tc.tile_pool(name="tmpp", bufs=6))

        qstate = {"qn": 0, "last": {}}

        def emit_p1(b):
            table = tables[b]

            # er for the core's own nodes (A order and B order)
            for src_t, er_dst in ((fLT, erA_sbs[b]), (fLTB, erB_sbs[b])):
                fl = slabp.tile([P, T * P], bf16, tag="fl")
                nc.sync.dma_start(fl[:], src_t)
                for t in range(T):
                    pse = psp.tile([P, 4], f32, tag="pse")
                    nc.tensor.matmul(out=pse[:],
                                     lhsT=fl[:, t * P : (t + 1) * P],
                                     rhs=wr_sb[:], start=True, stop=True)
                    nc.vector.tensor_copy(er_dst[:, t, :], pse[:])

            # table build: region A is fT tiles [0, NT_A), region B the rest.
            # fT column j*128+p holds the node of table row base + p*NT + j,
            # so partition p's stage row j is table row p*NT+j: the write is
            # one contiguous (nt*256B) descriptor per partition.
            def build_region(base_row, nt_region, ft_lo):
                t0 = 0
                while t0 < nt_region:
                    nt = min(slab_tiles, nt_region - t0)
                    slab = slabp.tile([P, slab_tiles * P], bf16, tag="slab")
                    nc.sync.dma_start(
                        slab[:, : nt * P],
                        fT[:, (ft_lo + t0) * P : (ft_lo + t0 + nt) * P])
                    stage = stagep.tile([P, slab_tiles, ROW], bf16,
                                        tag="stage")
                    for j in range(nt):
                        ps = psp.tile([P, 68], f32, tag="ps")
                        nc.tensor.matmul(out=ps[:],
                                         lhsT=slab[:, j * P : (j + 1) * P],
                                         rhs=waug_sb[:], start=True, stop=True)
                        nc.scalar.activation(stage[:, j, 0:64], ps[:, 0:64],
                                             Act.Copy)
                        nc.scalar.activation(
                            stage[:, j, 64:72].bitcast(f32), ps[:, 64:68],
                            Act.Copy)
                    # partition p -> rows [base + p*nt_region + t0, +nt);
                    # writes ride the ACT HWDGE ring so they don't block the
                    # next slab load (SP ring) behind their stage dependency
                    dst = table[base_row : base_row + P * nt_region, :] \
                        .rearrange("(p j) f -> p j f", p=P)
                    nc.scalar.dma_start(
                        dst[:, t0 : t0 + nt, :], stage[:, :nt, :])
                    t0 += nt

            build_region(0, NT_A, 0)
            build_region(plan.B_BASE, NT_B, NT_A)

        def emit_p2(b):
            table = tables[b]
            tabA = table[0:A_CAP, :]
            tabB = table[plan.B_BASE : TROWS, :]
            flushed = [False]
            for ui, u in enumerate(plan.units):
                if SKIP_P2:
                    break
                t = u["t"]
                C = u["cols"]
                isA_u = u["side"] == "A"
                if not isA_u and not flushed[0]:
                    # end of A pass: flush all A accumulators (cast f32->bf16)
                    # to DRAM in one contiguous slot-major DMA for the combine
                    flushed[0] = True
                    if not (SKIP_GATHER or SKIP_COMPUTE):
                        nc.vector.tensor_copy(accAbf[:, :, 0:68], accA_all[:])
                    nc.scalar.dma_start(
                        accAd.rearrange("(p t) f -> p t f", p=P), accAbf[:])
                if not isA_u and u["first"] and t % plan.CCH == 0:
                    # A accumulators (bf16 rows in accAd) -> B slot order
                    j = t // plan.CCH
                    gi = nc.gpsimd.dma_gather(
                        G2[:, j * plan.CCH : (j + 1) * plan.CCH, :],
                        accAd,
                        comb_sb[:, j * 8 * plan.CCH : (j + 1) * 8 * plan.CCH],
                        plan.CCH * P, plan.CCH * P, ROW,
                        queue_num=qstate["qn"])
                    if CHAIN:
                        hist = qstate["last"].setdefault(qstate["qn"], [])
                        hist.append(gi.ins)
                        if len(hist) > DEPTH:
                            add_dep_helper(gi.ins, hist.pop(0),
                                           reason="swdge ring throttle")
                    qstate["qn"] = (qstate["qn"] + 1) % NQUEUES
                er_sb = erA_sbs[b] if isA_u else erB_sbs[b]
                acc_all = accA_all if isA_u else accB_all
                acc68 = acc_all[:, t, 0:68]

                def do_gathers(G):
                    for (side, gcol, k0, ck), o in zip(u["calls"],
                                                       plan.call_off[ui]):
                        tab = tabA if side == "A" else tabB
                        gi = nc.gpsimd.dma_gather(
                            G[:, gcol : gcol + ck, :], tab,
                            idx_sb[:, o : o + 8 * ck],
                            ck * P, ck * P, ROW, queue_num=qstate["qn"])
                        if CHAIN:
                            hist = qstate["last"].setdefault(qstate["qn"], [])
                            hist.append(gi.ins)
                            if len(hist) > DEPTH:
                                add_dep_helper(gi.ins, hist.pop(0),
                                               reason="swdge ring throttle")
                        qstate["qn"] = (qstate["qn"] + 1) % NQUEUES

                if SKIP_GATHER or SKIP_COMPUTE:
                    G = gp.tile([P, cmax, ROW], bf16, tag="G")
                    if not SKIP_GATHER:
                        do_gathers(G)
                    if u["last"] and not isA_u:
                        fin = tmpp.tile([P, 16], f32, tag="fin")
                        if SKIP_GATHER:
                            nc.vector.memset(fin[:], 0.0)
                        else:
                            nc.vector.tensor_reduce(
                                fin[:], G[:, :max(C, 1), 0:16]
                                .rearrange("p c f -> p f c"),
                                mybir.AxisListType.X, Alu.add)
                        nc.vector.tensor_copy(outbuf[:, t, :], fin[:])
                    continue
                if C > 0:
                    G = gp.tile([P, cmax, ROW], bf16, tag="G")
                    do_gathers(G)
                    es = esp.tile([P, cmax, 4], f32, tag="es")
                    ee = eep.tile([P, cmax, 4], f32, tag="ee")
                    # e = el_src + er_dst, leaky-relu (small SBUF ops)
                    nc.vector.tensor_tensor(
                        es[:, :C, :],
                        G[:, :C, EL_OFF : EL_OFF + 8].bitcast(f32),
                        er_sb[:, t : t + 1, :].to_broadcast([P, C, 4]),
                        Alu.add)
                    nc.vector.scalar_tensor_tensor(
                        es[:, :C, :], es[:, :C, :], NEG_SLOPE, es[:, :C, :],
                        Alu.mult, Alu.max)
                    # exp lands in PSUM so the big mult's second operand
                    # stays off the shared SBUF port pair
                    nc.scalar.activation(ee[:, :C, :], es[:, :C, :], Act.Exp)
                    # msg = feat * ee (ee broadcast from PSUM: keeps the
                    # shared SBUF port free for SWDGE descriptor writes)
                    if not NOMULT:
                        nc.vector.tensor_tensor(
                            G[:, :C, 0:FEAT].rearrange("p c (h d) -> p c h d",
                                                       h=HEADS),
                            G[:, :C, 0:FEAT].rearrange("p c (h d) -> p c h d",
                                                       h=HEADS),
                            ee[:, :C, :].unsqueeze(-1)
                            .to_broadcast([P, C, HEADS, HID]),
                            Alu.mult)
                    # k-reductions: msg from SBUF (1-input), ee from PSUM
                    msum = G[:, :C, 0:FEAT].rearrange("p c f -> p f c")
                    dsum = ee[:, :C, :].rearrange("p c h -> p h c")
                    if u["first"]:
                        nc.vector.tensor_reduce(
                            acc68[:, 0:FEAT], msum, mybir.AxisListType.X,
                            Alu.add)
                        nc.vector.tensor_reduce(
                            acc68[:, FEAT : FEAT + 4], dsum,
                            mybir.AxisListType.X, Alu.add)
                    else:
                        tmp = tmpp.tile([P, 68], f32, tag="tmp")
                        nc.vector.tensor_reduce(
                            tmp[:, 0:FEAT], msum, mybir.AxisListType.X,
                            Alu.add)
                        nc.vector.tensor_reduce(
                            tmp[:, FEAT : FEAT + 4], dsum,
                            mybir.AxisListType.X, Alu.add)
                        nc.vector.tensor_add(acc68, acc68, tmp[:])
                elif u["first"]:
                    nc.vector.memset(acc68, 0.0)
                if u["last"] and not isA_u:
                    tmp68 = tmpp.tile([P, 68], f32, tag="tmp68")
                    nc.vector.tensor_tensor(
                        tmp68[:], G2[:, t, 0:68], acc68, Alu.add)
                    dent = tmp68[:, FEAT : FEAT + 4]
                    nc.vector.tensor_scalar_max(dent, dent, 1e-30)
                    rec = tmpp.tile([P, 4], f32, tag="rec")
                    nc.vector.reciprocal(rec[:], dent)
                    nc.vector.tensor_tensor(
                        tmp68[:, 0:FEAT].rearrange("p (h d) -> p h d",
                                                   h=HEADS),
                        tmp68[:, 0:FEAT].rearrange("p (h d) -> p h d",
                                                   h=HEADS),
                        rec[:].unsqueeze(-1).to_broadcast([P, HEADS, HID]),
                        Alu.mult)
                    fin = tmpp.tile([P, 16], f32, tag="fin")
                    nc.vector.tensor_reduce(
                        fin[:],
                        tmp68[:, 0:FEAT].rearrange("p (h d) -> p d h",
                                                   h=HEADS),
                        mybir.AxisListType.X, Alu.add)
                    nc.vector.scalar_tensor_tensor(
                        outbuf[:, t, :], fin[:], 1.0 / HEADS, biasm_sb[:],
                        Alu.mult, Alu.add)
            if not SKIP_P2:
                # slot-major output: out row p*T + t <- outbuf[p, t, :]
                nc.sync.dma_start(
                    out.rearrange("(p t) f -> p t f", p=P), outbuf[:])

        if reps > 1:
            R = reps - (reps % 2)
            emit_p1(0)
            if R // 2 - 1 > 0:
                with tc.For_i(0, R // 2 - 1, 1):
                    emit_p1(1)
                    emit_p2(0)
                    emit_p1(0)
                    emit_p2(1)
            emit_p1(1)
            emit_p2(0)
            emit_p2(1)
            if reps % 2:
                emit_p1(0)
                emit_p2(0)
        else:
            emit_p1(0)
            emit_p2(0)

    nc.compile()
    return nc


# ---------------- host wrapper ----------------

_CACHE = {}


def _get_plan_and_nc(src, dst, n_nodes, n_edges, ncores, cmax=CMAX,
                     slab_tiles=SLAB_TILES):
    key = (int(src[0]), int(src[-1]), int(dst[0]), int(dst[-1]),
           len(src), n_nodes, ncores, cmax)
    if key not in _CACHE:
        plan = make_plan(src, dst, n_nodes, n_edges, ncores, cmax=cmax)
        nc = build_nc(plan, ncores, slab_tiles=slab_tiles, cmax=cmax)
        _CACHE[key] = (plan, nc)
    return _CACHE[key]


def make_inputs(plan, features, W, attn_l, attn_r, bias):
    """Build per-core input maps from full inputs + plan."""
    import ml_dtypes

    bf16 = ml_dtypes.bfloat16
    features = np.asarray(features, dtype=np.float32)
    W = np.asarray(W, dtype=np.float32)
    attn_l = np.asarray(attn_l, dtype=np.float32)
    attn_r = np.asarray(attn_r, dtype=np.float32)
    bias = np.asarray(bias, dtype=np.float32)

    # augmented weights
    W3 = W.reshape(IN_DIM, HEADS, HID)
    Wl = np.einsum("ihd,hd->ih", W3, attn_l).astype(np.float32)
    Wr_ = np.einsum("ihd,hd->ih", W3, attn_r).astype(bf16)
    Waug = np.concatenate([W, Wl], axis=1).astype(bf16)  # [128, 68]
    biasm = bias.reshape(HEADS, HID).mean(axis=0).reshape(1, 16)
    biasm = np.ascontiguousarray(biasm, dtype=np.float32)

    # fT: [128, NFT*128] columns in table-build order (ft_node), zeros for pad
    ftn = plan.ft_node
    fT = np.zeros((IN_DIM, len(ftn)), dtype=bf16)
    real = ftn >= 0
    fT[:, np.nonzero(real)[0]] = features[ftn[real]].T.astype(bf16)
    fT = np.ascontiguousarray(fT)

    in_maps = []
    for c in range(plan.ncores):
        fLT = np.zeros((IN_DIM, plan.L), dtype=bf16)
        o = plan.order[c]
        real = o >= 0
        fLT[:, np.nonzero(real)[0]] = features[o[real]].T.astype(bf16)
        fLTB = np.zeros((IN_DIM, plan.L), dtype=bf16)
        ob = plan.orderB[c]
        realb = ob >= 0
        fLTB[:, np.nonzero(realb)[0]] = features[ob[realb]].T.astype(bf16)
        in_maps.append({
            "fT": fT,
            "fLT": np.ascontiguousarray(fLT),
            "fLTB": np.ascontiguousarray(fLTB),
            "Waug": Waug,
            "Wr": Wr_,
            "idx": np.ascontiguousarray(plan.idx_full[c]),
            "comb": np.ascontiguousarray(plan.comb_full[c]),
            "biasm": biasm,
        })
    return in_maps


def unshard_output(plan, outs):
    """outs: list of per-core {'out': [T*128,16]} -> full [N,16].
    Slot-major: node at orderB[c][t*128+p] lives at out row p*T + t."""
    res = np.empty((plan.N, 16), dtype=np.float32)
    T = plan.T
    for c in range(plan.ncores):
        o = plan.orderB[c]
        real = np.nonzero(o >= 0)[0]
        t = real // P
        p = real % P
        res[o[real]] = outs[c]["out"][p * T + t]
    return res


def kernel(features, W, attn_l, attn_r, bias, src, dst):
    from concourse.bass_utils import run_bass_kernel_spmd

    src = np.asarray(src)
    dst = np.asarray(dst)
    plan, nc = _get_plan_and_nc(src, dst, N_NODES, N_EDGES, NCORES)
    in_maps = make_inputs(plan, features, W, attn_l, attn_r, bias)
    res = run_bass_kernel_spmd(nc, in_maps, core_ids=list(range(NCORES)))
    return unshard_output(plan, res.results)
